# revision 1
# baseline (speedup 1.0000x reference)
"""Trainium2 Bass kernel for the CMDF block (dense_cnn).

Contract: kernel(**inputs) takes the FULL unsharded inputs (B=8, C=128,
H=W=64) and returns the FULL (8, 128, 64, 64) float32 output.

Sharding: data-parallel over batch — core b computes batch element b.
All weights are replicated (host-side prepacked into matmul layouts).

Math per batch element (see reference):
  Xs   = depthwise3x3(X2, static_w)
  ctx  = relu(w2 @ (w1 @ mean_hw([Xs; Y2])))
  cf   = (w3 @ ctx).reshape(C, 9)          # per-channel dynamic filter
  sf   = ws @ [Xs; Y2]                     # (9, H, W) spatial filter
  dyn  = sum_k shift_k(X2) * (cf[:, k] + sf[k])
  out  = wf[:, :C] @ Xs + wf[:, C:] @ dyn

Kernel strategy (channels on partitions, pixels on the free dim):
  - Xs via 9 accumulating PE matmuls with diag(sw[:, k]) weights over a
    zero-padded X held in SBUF. All large matmuls run in fp32r (full-rate
    fp32 mode, 11-bit mantissa); operands are pre-rounded on the host or
    rounded on-chip by their producing ACT/DVE instruction.
  - sf via matmuls with M=105 (ws replicated into 4 row-groups so the
    per-tap partition-broadcast matmuls can be row-tiled).
  - per tap k: broadcast sf[k] to 128 partitions with a 0/1 "selector"
    matmul, then ONE fused DVE op P_k = (sf_bc + cf[:,k]) * shift_k(X),
    then an accumulating matmul out += wfbT.T @ P_k. The sum over taps
    happens inside the final conv's PSUM accumulation.
"""

import numpy as np

import concourse.bass as bass
import concourse.tile as tile
import concourse.mybir as mybir
from concourse.bass_utils import run_bass_kernel_spmd

B, C, H, W, K = 8, 128, 64, 64, 3
HW = H * W            # 4096
PH, PW = H + 2, W + 2  # 66, 66 padded
NST = 4               # super-tiles over rows
ROWS = H // NST       # 16 image rows per super-tile
STN = ROWS * W        # 1024 pixels per super-tile (2 PSUM banks)
NT = K * K            # 9 taps
MREP = 3 * 32 + NT    # 105: ws replicated at partition groups 0,32,64,96

F32 = mybir.dt.float32
F32R = mybir.dt.float32r
ADD = mybir.AluOpType.add
MULT = mybir.AluOpType.mult
AX = mybir.AxisListType
ACT_COPY = mybir.ActivationFunctionType.Copy
ACT_RELU = mybir.ActivationFunctionType.Relu

_CACHE = {}


def round_f32r(a):
    """Round fp32 to fp32r (RNE at mantissa bit 12) — matches the
    walrus cast_fp32_to_fp32r used by the FP32r matmul datapath."""
    u = np.ascontiguousarray(a, dtype=np.float32).view(np.uint32).astype(np.uint64)
    r = ((u + 0x7FF + ((u >> 12) & 1)) & 0xFFFFF000).astype(np.uint32)
    return r.view(np.float32).reshape(np.asarray(a).shape)


BF16 = mybir.dt.bfloat16


def _absorb(nc, dep_elem, ps_elem):
    """Tiny bf16 matmul that reads one element of `dep_elem` and writes a
    junk element of `ps_elem` (later overwritten by a start=True group).
    Purpose: acquire the semaphore wait on dep_elem's producer on a plain
    (non-fused) matmul, so the following fused f32r matmul — which can
    embed only ONE sem wait — doesn't need two."""
    lh = dep_elem.bitcast(BF16)
    nc.tensor.matmul(ps_elem, lh[:, 0:1], lh[:, 0:1], start=True, stop=True)


def _split_multiwaits(nc):
    """walrus codegen in this toolchain accepts only ONE embedded sem wait
    per instruction. Hoist excess waits onto same-engine NoOps placed
    immediately before the instruction (engines execute in order, so the
    blocking behavior is identical)."""
    ctr = 0
    for fn in nc.m.functions:
        for blk in fn.blocks:
            insts = blk.instructions
            out = []
            for inst in insts:
                si = inst.sync_info
                waits = list(si.on_wait) if si is not None and si.on_wait else []
                if len(waits) > 1:
                    for w in waits[:-1]:
                        ctr += 1
                        out.append(mybir.InstNoOp(
                            name=f"I-wsplit-{ctr}",
                            engine=inst.engine,
                            ins=[], outs=[],
                            sync_info=mybir.SyncInfo(
                                on_wait=[w], on_update=[]),
                        ))
                    inst.sync_info = mybir.SyncInfo(
                        on_wait=[waits[-1]],
                        on_update=list(si.on_update) if si.on_update else [],
                    )
                out.append(inst)
            blk.instructions = out


def _build_bass():
    nc = bass.Bass("TRN2", target_bir_lowering=False, debug=False)

    # single input pack: xpad | y2 | dsw | wsa | wsb | wfa | wfb | bct | w1ab | w2t+w3t
    # one DMA -> one producer proc -> every consumer needs at most one wait
    WR_COLS = NT * C + MREP + MREP + C + C + NT * C  # 2770
    PK_COLS = PH * PW + HW + WR_COLS + 2 * 64 + (64 + NT * C)
    pk = nc.dram_tensor("pk", [C, PK_COLS], F32R, kind="ExternalInput").ap()
    ob = nc.dram_tensor("ob", [C, H, W], F32, kind="ExternalOutput").ap()

    with tile.TileContext(nc) as tc:
        with tc.tile_pool(name="singles", bufs=1) as S:
            stg = S.tile([C, PK_COLS], F32R)
            o = 0
            xpad = stg[:, o : o + PH * PW].rearrange(
                "p (h w) -> p h w", w=PW); o += PH * PW
            y2 = stg[:, o : o + HW]; o += HW
            t_dsw = stg[:, o : o + NT * C]; o += NT * C
            t_wsa = stg[:, o : o + MREP]; o += MREP
            t_wsb = stg[:, o : o + MREP]; o += MREP
            t_wfa = stg[:, o : o + C]; o += C
            t_wfb = stg[:, o : o + C]; o += C
            t_bct = stg[:, o : o + NT * C]; o += NT * C
            t_w1a = stg[:, o : o + 64].bitcast(F32); o += 64
            t_w1b = stg[:, o : o + 64].bitcast(F32); o += 64
            t_w2t = stg[0:64, o : o + 64].bitcast(F32); o += 64
            t_w3t = stg[0:64, o : o + NT * C].bitcast(F32); o += NT * C
            assert o == PK_COLS
            xs = S.tile([C, HW], F32R)
            sfs = S.tile([MREP, HW], F32R)

            xs_parts = S.tile([C, NST], F32)
            y2sum = S.tile([C, 1], F32)
            xs_sum = S.tile([C, 1], F32)
            mxs = S.tile([C, 1], F32)
            my2 = S.tile([C, 1], F32)
            ctx1 = S.tile([64, 1], F32)
            ctx2 = S.tile([64, 1], F32)
            cfsb = S.tile([C, NT], F32)

            # split the input load across DMA queues (the wait-splitter
            # pass makes multi-producer fan-in legal)
            A = PH * PW
            Bc = PH * PW + HW
            nc.sync.dma_start(out=stg[:, 0:A], in_=pk[:, 0:A])
            nc.sync.dma_start(out=stg[:, A:Bc], in_=pk[:, A:Bc])
            nc.sync.dma_start(out=stg[:, Bc:], in_=pk[:, Bc:])

            # mean(Y2) ingredient — DVE is idle during phase A
            nc.vector.tensor_reduce(out=y2sum, in_=y2, axis=AX.X, op=ADD)

            # ---------- phase A: Xs (static depthwise) + sf ----------
            with tc.tile_pool(name="psA", bufs=2, space="PSUM") as psA, \
                 tc.tile_pool(name="psSF", bufs=2, space="PSUM") as psSF:
                for t in range(NST):
                    xs_ps = psA.tile([C, 2, 512], F32, tag="xs_ps")
                    for h in range(2):
                        for k in range(NT):
                            dh, dw = divmod(k, 3)
                            r0 = 16 * t + 8 * h + dh
                            rhs = xpad[:, r0 : r0 + 8, dw : dw + W]
                            nc.tensor.matmul(
                                xs_ps[:, h, :],
                                t_dsw[:, k * C : (k + 1) * C],
                                rhs,
                                start=(k == 0),
                                stop=(k == NT - 1),
                            )
                    nc.scalar.activation(
                        out=xs[:, t * STN : (t + 1) * STN],
                        in_=xs_ps,
                        func=ACT_COPY,
                        accum_out=xs_parts[:, t : t + 1],
                    )
                    sf_ps = psSF.tile([MREP, 2, 512], F32, tag="sf_ps")
                    _absorb(nc, xs[0:1, t * STN : t * STN + 1],
                            sf_ps[0:1, 0, 0:1])
                    for h in range(2):
                        c0 = t * STN + h * 512
                        nc.tensor.matmul(
                            sf_ps[:, h, :],
                            t_wsa,
                            xs[:, c0 : c0 + 512],
                            start=True,
                            stop=False,
                        )
                        nc.tensor.matmul(
                            sf_ps[:, h, :],
                            t_wsb,
                            y2[:, c0 : c0 + 512],
                            start=False,
                            stop=True,
                        )
                    nc.scalar.copy(
                        out=sfs[:, t * STN : (t + 1) * STN], in_=sf_ps
                    )

            # ---------- phase B: context branch -> cf ----------
            with tc.tile_pool(name="psCtx", bufs=1, space="PSUM") as psX:
                nc.vector.tensor_reduce(out=xs_sum, in_=xs_parts, axis=AX.X, op=ADD)
                nc.scalar.mul(out=mxs, in_=xs_sum, mul=1.0 / HW)
                nc.scalar.mul(out=my2, in_=y2sum, mul=1.0 / HW)

                ctx1_ps = psX.tile([64, 1], F32, tag="ctx1")
                _absorb(nc, mxs[0:1, 0:1], ctx1_ps[0:1, 0:1])
                nc.tensor.matmul(ctx1_ps, t_w1a, mxs, start=True, stop=False)
                nc.tensor.matmul(ctx1_ps, t_w1b, my2, start=False, stop=True)
                nc.scalar.copy(out=ctx1, in_=ctx1_ps)

                ctx2_ps = psX.tile([64, 1], F32, tag="ctx2")
                nc.tensor.matmul(ctx2_ps, t_w2t, ctx1, start=True, stop=True)
                nc.scalar.activation(out=ctx2, in_=ctx2_ps, func=ACT_RELU)

                cf_ps = psX.tile([C, NT], F32, tag="cf")
                for k in range(NT):
                    nc.tensor.matmul(
                        cf_ps[:, k : k + 1], t_w3t[:, k * C : (k + 1) * C],
                        ctx2, start=True, stop=True,
                    )
                nc.scalar.copy(out=cfsb, in_=cf_ps)

            # ---------- phase C: dynamic filter + fusion conv ----------
            with tc.tile_pool(name="psBC", bufs=2, space="PSUM") as psBC, \
                 tc.tile_pool(name="psOut", bufs=2, space="PSUM") as psO, \
                 tc.tile_pool(name="pP", bufs=3) as pP, \
                 tc.tile_pool(name="pOsb", bufs=2) as pOsb:
                for t in range(NST):
                    out_ps = psO.tile([C, 2, 8, W], F32, tag="out_ps")
                    _absorb(nc, xs[0:1, t * STN : t * STN + 1],
                            out_ps[0:1, 0, 0, 0:1])
                    for h in range(2):
                        c0 = t * STN + h * 512
                        nc.tensor.matmul(
                            out_ps[:, h],
                            t_wfa,
                            xs[:, c0 : c0 + 512],
                            start=True,
                            stop=False,
                        )
                    for k in range(NT):
                        g = k % 2
                        bc_ps = psBC.tile([C, ROWS, W], F32, tag="bc")
                        if k == 0:
                            _absorb(nc, sfs[0:1, t * STN : t * STN + 1],
                                    bc_ps[0:1, 0, 0:1])
                        for h in range(2):
                            c0 = t * STN + h * 512
                            nc.tensor.matmul(
                                bc_ps[:, 8 * h : 8 * h + 8, :],
                                t_bct[32 * g : 32 * g + NT,
                                      k * C : (k + 1) * C],
                                sfs[32 * g : 32 * g + NT, c0 : c0 + 512],
                                start=True,
                                stop=True,
                                tile_position=(32 * g, 0),
                            )
                        dh, dw = divmod(k, 3)
                        p_sb = pP.tile([C, ROWS, W], F32R, tag="p")
                        nc.vector.scalar_tensor_tensor(
                            out=p_sb,
                            in0=bc_ps,
                            scalar=cfsb[:, k : k + 1],
                            in1=xpad[:, 16 * t + dh : 16 * t + dh + ROWS,
                                     dw : dw + W],
                            op0=ADD,
                            op1=MULT,
                        )
                        for h in range(2):
                            nc.tensor.matmul(
                                out_ps[:, h],
                                t_wfb,
                                p_sb[:, 8 * h : 8 * h + 8, :],
                                start=False,
                                stop=(k == NT - 1),
                            )
                    o_sb = pOsb.tile([C, 2, 8, W], F32, tag="osb")
                    nc.scalar.copy(out=o_sb, in_=out_ps)
                    nc.sync.dma_start(
                        out=ob[:, 16 * t : 16 * t + 16, :],
                        in_=o_sb.rearrange("c b r w -> c (b r) w"),
                    )
    _split_multiwaits(nc)
    return nc


def _prep_weights(static_w, w1, w2, w3, ws, wf):
    """Repack the tiny weights into the SBUF layouts the kernel expects."""
    f = np.float32
    sw = np.ascontiguousarray(static_w.reshape(C, NT), dtype=f)

    dsw = np.zeros((C, NT * C), dtype=f)
    for k in range(NT):
        dsw[np.arange(C), k * C + np.arange(C)] = sw[:, k]

    wsa = np.zeros((C, MREP), dtype=f)
    wsb = np.zeros((C, MREP), dtype=f)
    for g in range(4):
        for k in range(NT):
            wsa[:, 32 * g + k] = ws[k, :C]
            wsb[:, 32 * g + k] = ws[k, C:]

    bct = np.zeros((C, NT * C), dtype=f)
    for g in range(4):
        for k in range(NT):
            bct[32 * g + k, k * C : (k + 1) * C] = 1.0

    wfa = np.ascontiguousarray(wf[:, :C].T, dtype=f)
    wfb = np.ascontiguousarray(wf[:, C:].T, dtype=f)
    wr = round_f32r(
        np.concatenate([dsw, wsa, wsb, wfa, wfb, bct], axis=1)
    )
    wfp = np.concatenate(
        [np.ascontiguousarray(w1[:, :C].T, dtype=f),
         np.ascontiguousarray(w1[:, C:].T, dtype=f)], axis=1
    )
    w3t = np.ascontiguousarray(
        w3.reshape(C, NT, 64).transpose(2, 1, 0), dtype=f
    ).reshape(64, NT * C)
    wg64 = np.concatenate(
        [np.ascontiguousarray(w2.T, dtype=f), w3t], axis=1
    )
    wg = np.zeros((C, wg64.shape[1]), dtype=f)
    wg[:64] = wg64
    return np.concatenate([wr, wfp, wg], axis=1)


def make_in_maps(X2, Y2, static_w, w1, w2, w3, ws, wf):
    wpack = _prep_weights(
        np.asarray(static_w), np.asarray(w1), np.asarray(w2),
        np.asarray(w3), np.asarray(ws), np.asarray(wf),
    )
    X2 = np.asarray(X2)
    Y2 = np.asarray(Y2)
    xpad_all = np.zeros((B, C, PH, PW), dtype=np.float32)
    xpad_all[:, :, 1 : H + 1, 1 : W + 1] = X2
    xpad_all = round_f32r(xpad_all).reshape(B, C, PH * PW)
    y2_all = round_f32r(Y2.reshape(B, C, HW))
    in_maps = []
    for b in range(B):
        m = {"pk": np.ascontiguousarray(np.concatenate(
            [xpad_all[b], y2_all[b], wpack], axis=1))}
        in_maps.append(m)
    return in_maps


def get_nc():
    if "nc" not in _CACHE:
        _CACHE["nc"] = _build_bass()
    return _CACHE["nc"]


def kernel(X2, Y2, static_w, w1, w2, w3, ws, wf):
    nc = get_nc()
    in_maps = make_in_maps(
        np.asarray(X2), np.asarray(Y2), static_w, w1, w2, w3, ws, wf
    )
    res = run_bass_kernel_spmd(nc, in_maps, core_ids=list(range(B)))
    out = np.stack([r["ob"] for r in res.results]).astype(np.float32)
    return out



# revision 2
# speedup vs baseline: 1.0350x; 1.0350x over previous
"""Trainium2 Bass kernel for the CMDF block (dense_cnn), V3.

Contract: kernel(**inputs) takes the FULL unsharded inputs (B=8, C=128,
H=W=64) and returns the FULL (8, 128, 64, 64) float32 output.

Sharding: data-parallel over batch - core b computes batch element b.
All weights are replicated (host-side prepacked into matmul layouts).

Math per batch element (see reference):
  Xs   = depthwise3x3(X2, static_w)
  ctx  = relu(w2 @ (w1 @ mean_hw([Xs; Y2])))
  cf   = (w3 @ ctx).reshape(C, 9)          # per-channel dynamic filter
  sf   = ws @ [Xs; Y2]                     # (9, H, W) spatial filter
  dyn  = sum_k shift_k(X2) * (cf[:, k] + sf[k])
  out  = wf[:, :C] @ Xs + wf[:, C:] @ dyn

V3 design notes (vs the f32r V1):
  - Everything streams in bf16 (3.2MB input vs 6.4MB): DMA lead-in halves.
    Weights land first (chunked DMA) so PE starts at ~3us, not ~21us.
  - mean_hw(Xs) is computed WITHOUT Xs: mean of a zero-padded shifted
    image = S - edge strips + corners, so a per-tap host-precomputed
    coefficient vector turns 9 shifted means into one stt chain over
    {S, row0, row63, col0, col63, 4 corners}. The context branch then
    runs concurrently with phase A instead of serializing after it.
  - Phase C taps are split across engines: 6 taps as DVE stt reading the
    broadcast filter from PSUM; 3 taps via ACT Identity-with-bias copy
    (PSUM -> SBUF bf16, folds cf in) followed by a Pool tensor_tensor
    multiply. PE does the 9 broadcast + 9 fusion matmuls per tile and is
    the ~8.1us/tile bottleneck; DVE/ACT/Pool hide underneath.
"""

import numpy as np
import ml_dtypes

import concourse.bass as bass
import concourse.tile as tile
import concourse.mybir as mybir
from concourse.bass_utils import run_bass_kernel_spmd

B, C, H, W, K = 8, 128, 64, 64, 3
HW = H * W            # 4096
PH, PW = H + 2, W + 2  # 66, 66 padded
NST = 4               # super-tiles over rows
ROWS = H // NST       # 16 image rows per super-tile
STN = ROWS * W        # 1024 pixels per super-tile (2 PSUM banks)
NT = K * K            # 9 taps
MREP = 3 * 32 + NT    # 105: ws replicated at partition groups 0,32,64,96

F32 = mybir.dt.float32
BF16 = mybir.dt.bfloat16
ADD = mybir.AluOpType.add
MULT = mybir.AluOpType.mult
AX = mybir.AxisListType
ACT_COPY = mybir.ActivationFunctionType.Copy
ACT_IDENT = mybir.ActivationFunctionType.Identity
ACT_RELU = mybir.ActivationFunctionType.Relu

BFDT = ml_dtypes.bfloat16

# taps handled by ACT(bias)+Pool(mult); the rest are DVE stt
ACT_TAPS = (0, 1, 2)

# ---- pk column layout (bf16 columns) ----
O_DSW = 0
O_WSA = O_DSW + NT * C        # 1152
O_WSB = O_WSA + MREP          # 1257
O_W1A = O_WSB + MREP          # 1362
O_W1B = O_W1A + 64            # 1426
O_W2T = O_W1B + 64            # 1490
O_W3T = O_W2T + 64            # 1554
O_MC = O_W3T + NT * C         # 2706 (9 f32 coeffs = 18 bf16 cols)
O_XP = O_MC + 18              # 2724
O_Y2 = O_XP + PH * PW         # 7080
O_WFA = O_Y2 + HW             # 11176
O_WFB = O_WFA + C             # 11304
O_BCT = O_WFB + C             # 11432
PK_COLS = O_BCT + NT * C      # 12584

_CACHE = {}


def _absorb(nc, dep_elem, ps_elem):
    """Tiny bf16 matmul that reads one element of `dep_elem` and writes a
    junk element of `ps_elem` (later overwritten by a start=True group).
    Purpose: acquire the semaphore wait on dep_elem's producer on a plain
    (non-fused) matmul, so the following fused matmul - which can embed
    only ONE sem wait - doesn't need two."""
    lh = dep_elem
    nc.tensor.matmul(ps_elem, lh[:, 0:1], lh[:, 0:1], start=True, stop=True)


def _split_multiwaits(nc):
    """walrus codegen in this toolchain accepts only ONE embedded sem wait
    per instruction. Hoist excess waits onto same-engine NoOps placed
    immediately before the instruction (engines execute in order, so the
    blocking behavior is identical)."""
    ctr = 0
    for fn in nc.m.functions:
        for blk in fn.blocks:
            insts = blk.instructions
            out = []
            for inst in insts:
                si = inst.sync_info
                waits = list(si.on_wait) if si is not None and si.on_wait else []
                if len(waits) > 1:
                    for w in waits[:-1]:
                        ctr += 1
                        out.append(mybir.InstNoOp(
                            name=f"I-wsplit-{ctr}",
                            engine=inst.engine,
                            ins=[], outs=[],
                            sync_info=mybir.SyncInfo(
                                on_wait=[w], on_update=[]),
                        ))
                    inst.sync_info = mybir.SyncInfo(
                        on_wait=[waits[-1]],
                        on_update=list(si.on_update) if si.on_update else [],
                    )
                out.append(inst)
            blk.instructions = out


def _build_bass():
    nc = bass.Bass("TRN2", target_bir_lowering=False, debug=False)

    pk = nc.dram_tensor("pk", [C, PK_COLS], BF16, kind="ExternalInput").ap()
    ob = nc.dram_tensor("ob", [C, H, W], F32, kind="ExternalOutput").ap()

    with tile.TileContext(nc) as tc:
        with tc.tile_pool(name="singles", bufs=1) as S:
            stg = S.tile([C, PK_COLS], BF16)
            t_dsw = stg[:, O_DSW:O_DSW + NT * C]
            t_wsa = stg[:, O_WSA:O_WSA + MREP]
            t_wsb = stg[:, O_WSB:O_WSB + MREP]
            t_w1a = stg[:, O_W1A:O_W1A + 64]
            t_w1b = stg[:, O_W1B:O_W1B + 64]
            t_w2t = stg[0:64, O_W2T:O_W2T + 64]
            t_w3t = stg[0:64, O_W3T:O_W3T + NT * C]
            t_mc = stg[:, O_MC:O_MC + 18].bitcast(F32)  # [C, 9] f32
            xpad = stg[:, O_XP:O_XP + PH * PW].rearrange(
                "p (h w) -> p h w", w=PW)
            xpflat = stg[:, O_XP:O_XP + PH * PW]
            y2 = stg[:, O_Y2:O_Y2 + HW]
            t_wfa = stg[:, O_WFA:O_WFA + C]
            t_wfb = stg[:, O_WFB:O_WFB + C]
            t_bct = stg[:, O_BCT:O_BCT + NT * C]

            xs = S.tile([C, HW], BF16)
            sfs = S.tile([MREP, HW], BF16)

            xpart = S.tile([C, 4], F32)
            y2part = S.tile([C, 2], F32)
            sx = S.tile([C, 1], F32)
            y2sum = S.tile([C, 1], F32)
            edge = S.tile([C, 4], F32)   # R0, R63, C0, C63
            accs = S.tile([C, 8], F32)
            mxs = S.tile([C, 1], BF16)
            my2 = S.tile([C, 1], BF16)
            ctx1 = S.tile([64, 1], BF16)
            ctx2 = S.tile([64, 1], BF16)
            cfsb = S.tile([C, NT], F32)

            # ---------- input DMAs: weights first, then chunked image ----
            XPB = [0, 18 * PW, 34 * PW, 50 * PW, PH * PW]
            nc.sync.dma_start(out=stg[:, 0:O_XP], in_=pk[:, 0:O_XP])
            for i in range(4):
                a, b = O_XP + XPB[i], O_XP + XPB[i + 1]
                nc.sync.dma_start(out=stg[:, a:b], in_=pk[:, a:b])
            for i in range(2):
                a, b = O_Y2 + 2048 * i, O_Y2 + 2048 * (i + 1)
                nc.sync.dma_start(out=stg[:, a:b], in_=pk[:, a:b])
            nc.sync.dma_start(out=stg[:, O_WFA:], in_=pk[:, O_WFA:])

            # ---------- means from X2/Y2 directly (DVE, overlaps A) ------
            for i in range(4):
                nc.vector.tensor_reduce(
                    out=xpart[:, i:i + 1],
                    in_=xpflat[:, XPB[i]:XPB[i + 1]], axis=AX.X, op=ADD)
            nc.vector.tensor_reduce(out=sx, in_=xpart, axis=AX.X, op=ADD)
            for i in range(2):
                nc.vector.tensor_reduce(
                    out=y2part[:, i:i + 1],
                    in_=y2[:, 2048 * i:2048 * (i + 1)], axis=AX.X, op=ADD)
            nc.vector.tensor_reduce(out=y2sum, in_=y2part, axis=AX.X, op=ADD)
            nc.vector.tensor_scalar(
                out=my2, in0=y2sum, scalar1=1.0 / HW, scalar2=None, op0=MULT)
            # edge strips (row 0/63, col 0/63 of the unpadded image)
            nc.vector.tensor_reduce(
                out=edge[:, 0:1], in_=xpad[:, 1, 1:65], axis=AX.X, op=ADD)
            nc.vector.tensor_reduce(
                out=edge[:, 1:2], in_=xpad[:, 64, 1:65], axis=AX.X, op=ADD)
            nc.vector.tensor_reduce(
                out=edge[:, 2:3], in_=xpad[:, 1:65, 1], axis=AX.X, op=ADD)
            nc.vector.tensor_reduce(
                out=edge[:, 3:4], in_=xpad[:, 1:65, 64], axis=AX.X, op=ADD)
            # stt chain: mxs = sum_i coef_i * term_i  (coefs carry sign+1/HW)
            terms = [edge[:, 0:1], edge[:, 1:2], edge[:, 2:3], edge[:, 3:4],
                     xpad[:, 1, 1:2], xpad[:, 1, 64:65],
                     xpad[:, 64, 1:2], xpad[:, 64, 64:65]]
            nc.vector.tensor_scalar(
                out=accs[:, 0:1], in0=sx, scalar1=t_mc[:, 0:1], scalar2=None,
                op0=MULT)
            for i, term in enumerate(terms):
                dst = accs[:, i + 1:i + 2] if i < 7 else mxs
                nc.vector.scalar_tensor_tensor(
                    out=dst, in0=term, scalar=t_mc[:, i + 1:i + 2],
                    in1=accs[:, i:i + 1], op0=MULT, op1=ADD)

            # ---------- context branch -> cf (overlaps phase A) ----------
            with tc.tile_pool(name="psCtx", bufs=1, space="PSUM") as psX:
                ctx1_ps = psX.tile([64, 1], F32, tag="ctx1")
                _absorb(nc, mxs, ctx1_ps[0:1, 0:1])
                nc.tensor.matmul(ctx1_ps, t_w1a, mxs, start=True, stop=False)
                nc.tensor.matmul(ctx1_ps, t_w1b, my2, start=False, stop=True)
                nc.scalar.copy(out=ctx1, in_=ctx1_ps)

                ctx2_ps = psX.tile([64, 1], F32, tag="ctx2")
                nc.tensor.matmul(ctx2_ps, t_w2t, ctx1, start=True, stop=True)
                nc.scalar.activation(out=ctx2, in_=ctx2_ps, func=ACT_RELU)

                cf_ps = psX.tile([C, NT], F32, tag="cf")
                for k in range(NT):
                    nc.tensor.matmul(
                        cf_ps[:, k:k + 1], t_w3t[:, k * C:(k + 1) * C],
                        ctx2, start=True, stop=True)
                nc.scalar.copy(out=cfsb, in_=cf_ps)

            # ---------- phase A: Xs (static depthwise) + sf --------------
            with tc.tile_pool(name="psA", bufs=2, space="PSUM") as psA, \
                 tc.tile_pool(name="psSF", bufs=1, space="PSUM") as psSF:
                for t in range(NST):
                    xs_ps = psA.tile([C, 2, 512], F32, tag="xs_ps")
                    for h in range(2):
                        for k in range(NT):
                            dh, dw = divmod(k, 3)
                            r0 = 16 * t + 8 * h + dh
                            rhs = xpad[:, r0:r0 + 8, dw:dw + W]
                            nc.tensor.matmul(
                                xs_ps[:, h, :],
                                t_dsw[:, k * C:(k + 1) * C],
                                rhs,
                                start=(k == 0),
                                stop=(k == NT - 1),
                            )
                    nc.scalar.copy(
                        out=xs[:, t * STN:(t + 1) * STN], in_=xs_ps)
                    sf_ps = psSF.tile([MREP, 2, 512], F32, tag="sf_ps")
                    _absorb(nc, xs[0:1, t * STN:t * STN + 1],
                            sf_ps[0:1, 0, 0:1])
                    for h in range(2):
                        c0 = t * STN + h * 512
                        nc.tensor.matmul(
                            sf_ps[:, h, :], t_wsa, xs[:, c0:c0 + 512],
                            start=True, stop=False)
                        nc.tensor.matmul(
                            sf_ps[:, h, :], t_wsb, y2[:, c0:c0 + 512],
                            start=False, stop=True)
                    nc.scalar.copy(
                        out=sfs[:, t * STN:(t + 1) * STN], in_=sf_ps)

            # ---------- phase C: dynamic filter + fusion conv ------------
            with tc.tile_pool(name="psBC", bufs=2, space="PSUM") as psBC, \
                 tc.tile_pool(name="psOut", bufs=2, space="PSUM") as psO, \
                 tc.tile_pool(name="pP", bufs=3) as pP, \
                 tc.tile_pool(name="pDF", bufs=2) as pDF, \
                 tc.tile_pool(name="pOsb", bufs=2) as pOsb:
                for t in range(NST):
                    out_ps = psO.tile([C, 2, 8, W], F32, tag="out_ps")
                    _absorb(nc, xs[0:1, t * STN:t * STN + 1],
                            out_ps[0:1, 0, 0, 0:1])
                    for h in range(2):
                        c0 = t * STN + h * 512
                        nc.tensor.matmul(
                            out_ps[:, h], t_wfa, xs[:, c0:c0 + 512],
                            start=True, stop=False)
                    for k in range(NT):
                        g = k % 2
                        bc_ps = psBC.tile([C, ROWS, W], F32, tag="bc")
                        if k == 0:
                            _absorb(nc, sfs[0:1, t * STN:t * STN + 1],
                                    bc_ps[0:1, 0, 0:1])
                        for h in range(2):
                            c0 = t * STN + h * 512
                            nc.tensor.matmul(
                                bc_ps[:, 8 * h:8 * h + 8, :],
                                t_bct[32 * g:32 * g + NT,
                                      k * C:(k + 1) * C],
                                sfs[32 * g:32 * g + NT, c0:c0 + 512],
                                start=True, stop=True,
                                tile_position=(32 * g, 0),
                            )
                        dh, dw = divmod(k, 3)
                        xsl = xpad[:, 16 * t + dh:16 * t + dh + ROWS,
                                   dw:dw + W]
                        p_sb = pP.tile([C, ROWS, W], BF16, tag="p")
                        if k in ACT_TAPS:
                            df = pDF.tile([C, ROWS, W], BF16, tag="df")
                            nc.scalar.activation(
                                out=df, in_=bc_ps, func=ACT_IDENT,
                                bias=cfsb[:, k:k + 1])
                            nc.gpsimd.tensor_tensor(
                                out=p_sb, in0=df, in1=xsl, op=MULT)
                        else:
                            nc.vector.scalar_tensor_tensor(
                                out=p_sb, in0=bc_ps,
                                scalar=cfsb[:, k:k + 1], in1=xsl,
                                op0=ADD, op1=MULT)
                        for h in range(2):
                            nc.tensor.matmul(
                                out_ps[:, h], t_wfb,
                                p_sb[:, 8 * h:8 * h + 8, :],
                                start=False, stop=(k == NT - 1))
                    o_sb = pOsb.tile([C, 2, 8, W], F32, tag="osb")
                    nc.scalar.copy(out=o_sb, in_=out_ps)
                    nc.sync.dma_start(
                        out=ob[:, 16 * t:16 * t + 16, :],
                        in_=o_sb.rearrange("c b r w -> c (b r) w"),
                    )
    _split_multiwaits(nc)
    return nc


def _prep_weights(static_w, w1, w2, w3, ws, wf):
    """Repack the tiny weights into the bf16 SBUF layouts."""
    f = np.float32
    sw = np.ascontiguousarray(static_w.reshape(C, NT), dtype=f)

    dsw = np.zeros((C, NT * C), dtype=f)
    for k in range(NT):
        dsw[np.arange(C), k * C + np.arange(C)] = sw[:, k]

    wsa = np.zeros((C, MREP), dtype=f)
    wsb = np.zeros((C, MREP), dtype=f)
    for g in range(4):
        for k in range(NT):
            wsa[:, 32 * g + k] = ws[k, :C]
            wsb[:, 32 * g + k] = ws[k, C:]

    w1a = np.ascontiguousarray(w1[:, :C].T, dtype=f)
    w1b = np.ascontiguousarray(w1[:, C:].T, dtype=f)
    w2t64 = np.ascontiguousarray(w2.T, dtype=f)
    w3t64 = np.ascontiguousarray(
        w3.reshape(C, NT, 64).transpose(2, 1, 0), dtype=f).reshape(64, NT * C)
    w2t = np.zeros((C, 64), dtype=f)
    w2t[:64] = w2t64
    w3t = np.zeros((C, NT * C), dtype=f)
    w3t[:64] = w3t64

    # mean-correction coefficients (sign and 1/HW folded in):
    # terms: [S, R0, R63, C0, C63, X00, X0_63, X63_0, X63_63]
    mc = np.zeros((C, 9), dtype=f)
    mc[:, 0] = sw.sum(axis=1)
    mc[:, 1] = -(sw[:, 6] + sw[:, 7] + sw[:, 8])   # dh=+1 excludes row0
    mc[:, 2] = -(sw[:, 0] + sw[:, 1] + sw[:, 2])   # dh=-1 excludes row63
    mc[:, 3] = -(sw[:, 2] + sw[:, 5] + sw[:, 8])   # dw=+1 excludes col0
    mc[:, 4] = -(sw[:, 0] + sw[:, 3] + sw[:, 6])   # dw=-1 excludes col63
    mc[:, 5] = sw[:, 8]
    mc[:, 6] = sw[:, 6]
    mc[:, 7] = sw[:, 2]
    mc[:, 8] = sw[:, 0]
    mc *= 1.0 / HW
    mc_bf = np.ascontiguousarray(mc).view(np.uint16).view(BFDT)  # [C, 18]

    wfa = np.ascontiguousarray(wf[:, :C].T, dtype=f)
    wfb = np.ascontiguousarray(wf[:, C:].T, dtype=f)

    bct = np.zeros((C, NT * C), dtype=f)
    for g in range(4):
        for k in range(NT):
            bct[32 * g + k, k * C:(k + 1) * C] = 1.0

    wts_pre = np.concatenate(
        [dsw, wsa, wsb, w1a, w1b, w2t, w3t], axis=1).astype(BFDT)
    wts_pre = np.concatenate([wts_pre, mc_bf], axis=1)
    wts_post = np.concatenate([wfa, wfb, bct], axis=1).astype(BFDT)
    return wts_pre, wts_post


def make_in_maps(X2, Y2, static_w, w1, w2, w3, ws, wf):
    wts_pre, wts_post = _prep_weights(
        np.asarray(static_w), np.asarray(w1), np.asarray(w2),
        np.asarray(w3), np.asarray(ws), np.asarray(wf),
    )
    X2 = np.asarray(X2)
    Y2 = np.asarray(Y2)
    xpad_all = np.zeros((B, C, PH, PW), dtype=np.float32)
    xpad_all[:, :, 1:H + 1, 1:W + 1] = X2
    xpad_all = xpad_all.reshape(B, C, PH * PW).astype(BFDT)
    y2_all = Y2.reshape(B, C, HW).astype(BFDT)
    in_maps = []
    for b in range(B):
        m = {"pk": np.ascontiguousarray(np.concatenate(
            [wts_pre, xpad_all[b], y2_all[b], wts_post], axis=1))}
        in_maps.append(m)
    return in_maps


def get_nc():
    if "nc" not in _CACHE:
        _CACHE["nc"] = _build_bass()
    return _CACHE["nc"]


def kernel(X2, Y2, static_w, w1, w2, w3, ws, wf):
    nc = get_nc()
    in_maps = make_in_maps(
        np.asarray(X2), np.asarray(Y2), static_w, w1, w2, w3, ws, wf
    )
    res = run_bass_kernel_spmd(nc, in_maps, core_ids=list(range(B)))
    out = np.stack([np.asarray(r["ob"], dtype=np.float32)
                    for r in res.results])
    return out


# revision 12
# speedup vs baseline: 1.5240x; 1.4725x over previous
"""Trainium2 Bass kernel for the CMDF block (dense_cnn), V3.

Contract: kernel(**inputs) takes the FULL unsharded inputs (B=8, C=128,
H=W=64) and returns the FULL (8, 128, 64, 64) float32 output.

Sharding: data-parallel over batch - core b computes batch element b.
All weights are replicated (host-side prepacked into matmul layouts).

Math per batch element (see reference):
  Xs   = depthwise3x3(X2, static_w)
  ctx  = relu(w2 @ (w1 @ mean_hw([Xs; Y2])))
  cf   = (w3 @ ctx).reshape(C, 9)          # per-channel dynamic filter
  sf   = ws @ [Xs; Y2]                     # (9, H, W) spatial filter
  dyn  = sum_k shift_k(X2) * (cf[:, k] + sf[k])
  out  = wf[:, :C] @ Xs + wf[:, C:] @ dyn

V3 design notes (vs the f32r V1):
  - Everything streams in bf16 (3.2MB input vs 6.4MB): DMA lead-in halves.
    Weights land first (chunked DMA) so PE starts at ~3us, not ~21us.
  - mean_hw(Xs) is computed WITHOUT Xs: mean of a zero-padded shifted
    image = S - edge strips + corners, so a per-tap host-precomputed
    coefficient vector turns 9 shifted means into one stt chain over
    {S, row0, row63, col0, col63, 4 corners}. The context branch then
    runs concurrently with phase A instead of serializing after it.
  - Phase C taps are split across engines: 6 taps as DVE stt reading the
    broadcast filter from PSUM; 3 taps via ACT Identity-with-bias copy
    (PSUM -> SBUF bf16, folds cf in) followed by a Pool tensor_tensor
    multiply. PE does the 9 broadcast + 9 fusion matmuls per tile and is
    the ~8.1us/tile bottleneck; DVE/ACT/Pool hide underneath.
"""

import numpy as np
import ml_dtypes

import concourse.bass as bass
import concourse.tile as tile
import concourse.mybir as mybir
from concourse.bass_utils import run_bass_kernel_spmd

B, C, H, W, K = 8, 128, 64, 64, 3
HW = H * W            # 4096
PH, PW = H + 2, W + 2  # 66, 66 padded
NST = 4               # super-tiles over rows
ROWS = H // NST       # 16 image rows per super-tile
STN = ROWS * W        # 1024 pixels per super-tile (2 PSUM banks)
NT = K * K            # 9 taps
MREP = 3 * 32 + NT    # 105: ws replicated at partition groups 0,32,64,96

F32 = mybir.dt.float32
BF16 = mybir.dt.bfloat16
ADD = mybir.AluOpType.add
MULT = mybir.AluOpType.mult
AX = mybir.AxisListType
ACT_COPY = mybir.ActivationFunctionType.Copy
ACT_IDENT = mybir.ActivationFunctionType.Identity
ACT_RELU = mybir.ActivationFunctionType.Relu

BFDT = ml_dtypes.bfloat16

# taps handled by ACT(bias)+Pool(mult); the rest are DVE stt
ACT_TAPS = (0, 1, 2)

# ---- pk column layout (bf16 columns) ----
O_DSW = 0
O_WSA = O_DSW + NT * C        # 1152
O_WSB = O_WSA + MREP          # 1257
O_W1A = O_WSB + MREP          # 1362
O_W1B = O_W1A + 64            # 1426
O_W2T = O_W1B + 64            # 1490
O_W3T = O_W2T + 64            # 1554
O_MC = O_W3T + NT * C         # 2706 (9 f32 coeffs = 18 bf16 cols)
O_XP = O_MC + 18              # 2724
O_Y2 = O_XP + PH * PW         # 7080
O_WFA = O_Y2 + HW             # 11176
O_WFB = O_WFA + C             # 11304
O_BCT = O_WFB + C             # 11432
PK_COLS = O_BCT + NT * C      # 12584

_CACHE = {}


def _absorb(nc, dep_elem, ps_elem):
    """Tiny bf16 matmul that reads one element of `dep_elem` and writes a
    junk element of `ps_elem` (later overwritten by a start=True group).
    Purpose: acquire the semaphore wait on dep_elem's producer on a plain
    (non-fused) matmul, so the following fused matmul - which can embed
    only ONE sem wait - doesn't need two."""
    lh = dep_elem
    nc.tensor.matmul(ps_elem, lh[:, 0:1], lh[:, 0:1], start=True, stop=True)


def _split_multiwaits(nc):
    """walrus codegen in this toolchain accepts only ONE embedded sem wait
    per instruction. Hoist excess waits onto same-engine NoOps placed
    immediately before the instruction (engines execute in order, so the
    blocking behavior is identical)."""
    ctr = 0
    for fn in nc.m.functions:
        for blk in fn.blocks:
            insts = blk.instructions
            out = []
            for inst in insts:
                si = inst.sync_info
                waits = list(si.on_wait) if si is not None and si.on_wait else []
                if len(waits) > 1:
                    for w in waits[:-1]:
                        ctr += 1
                        out.append(mybir.InstNoOp(
                            name=f"I-wsplit-{ctr}",
                            engine=inst.engine,
                            ins=[], outs=[],
                            sync_info=mybir.SyncInfo(
                                on_wait=[w], on_update=[]),
                        ))
                    inst.sync_info = mybir.SyncInfo(
                        on_wait=[waits[-1]],
                        on_update=list(si.on_update) if si.on_update else [],
                    )
                out.append(inst)
            blk.instructions = out


def _build_bass():
    nc = bass.Bass("TRN2", target_bir_lowering=False, debug=False)

    pk = nc.dram_tensor("pk", [C, PK_COLS], BF16, kind="ExternalInput").ap()
    ob = nc.dram_tensor("ob", [C, H, W], F32, kind="ExternalOutput").ap()

    with tile.TileContext(nc) as tc:
        with tc.tile_pool(name="singles", bufs=1) as S:
            stg = S.tile([C, PK_COLS], BF16)
            t_dsw = stg[:, O_DSW:O_DSW + NT * C]
            t_wsa = stg[:, O_WSA:O_WSA + MREP]
            t_wsb = stg[:, O_WSB:O_WSB + MREP]
            t_w1a = stg[:, O_W1A:O_W1A + 64]
            t_w1b = stg[:, O_W1B:O_W1B + 64]
            t_w2t = stg[0:64, O_W2T:O_W2T + 64]
            t_w3t = stg[0:64, O_W3T:O_W3T + NT * C]
            t_mc = stg[:, O_MC:O_MC + 18].bitcast(F32)  # [C, 9] f32
            xpad = stg[:, O_XP:O_XP + PH * PW].rearrange(
                "p (h w) -> p h w", w=PW)
            xpflat = stg[:, O_XP:O_XP + PH * PW]
            y2 = stg[:, O_Y2:O_Y2 + HW]
            t_wfa = stg[:, O_WFA:O_WFA + C]
            t_wfb = stg[:, O_WFB:O_WFB + C]
            t_bct = stg[:, O_BCT:O_BCT + NT * C]

            xs = S.tile([C, HW], BF16)
            sfs = S.tile([MREP, HW], BF16)

            xpart = S.tile([C, 4], F32)
            y2part = S.tile([C, 2], F32)
            sx = S.tile([C, 1], F32)
            y2sum = S.tile([C, 1], F32)
            edge = S.tile([C, 4], F32)   # R0, R63, C0, C63
            accs = S.tile([C, 8], F32)
            mxs = S.tile([C, 1], BF16)
            my2 = S.tile([C, 1], BF16)
            ctx1 = S.tile([64, 1], BF16)
            ctx2 = S.tile([64, 1], BF16)
            cfsb = S.tile([C, NT], F32)

            # ---------- input DMAs: weights first, then chunked image ----
            XPB = [0, 18 * PW, 34 * PW, 50 * PW, PH * PW]
            nc.sync.dma_start(out=stg[:, 0:O_XP], in_=pk[:, 0:O_XP])
            for i in range(4):
                a, b = O_XP + XPB[i], O_XP + XPB[i + 1]
                nc.sync.dma_start(out=stg[:, a:b], in_=pk[:, a:b])
            for i in range(2):
                a, b = O_Y2 + 2048 * i, O_Y2 + 2048 * (i + 1)
                nc.sync.dma_start(out=stg[:, a:b], in_=pk[:, a:b])
            nc.sync.dma_start(out=stg[:, O_WFA:], in_=pk[:, O_WFA:])

            # ---------- means from X2/Y2 directly (DVE, overlaps A) ------
            for i in range(4):
                nc.vector.tensor_reduce(
                    out=xpart[:, i:i + 1],
                    in_=xpflat[:, XPB[i]:XPB[i + 1]], axis=AX.X, op=ADD)
            nc.vector.tensor_reduce(out=sx, in_=xpart, axis=AX.X, op=ADD)
            for i in range(2):
                nc.vector.tensor_reduce(
                    out=y2part[:, i:i + 1],
                    in_=y2[:, 2048 * i:2048 * (i + 1)], axis=AX.X, op=ADD)
            nc.vector.tensor_reduce(out=y2sum, in_=y2part, axis=AX.X, op=ADD)
            nc.vector.tensor_scalar(
                out=my2, in0=y2sum, scalar1=1.0 / HW, scalar2=None, op0=MULT)
            # edge strips (row 0/63, col 0/63 of the unpadded image)
            nc.vector.tensor_reduce(
                out=edge[:, 0:1], in_=xpad[:, 1, 1:65], axis=AX.X, op=ADD)
            nc.vector.tensor_reduce(
                out=edge[:, 1:2], in_=xpad[:, 64, 1:65], axis=AX.X, op=ADD)
            nc.vector.tensor_reduce(
                out=edge[:, 2:3], in_=xpad[:, 1:65, 1], axis=AX.X, op=ADD)
            nc.vector.tensor_reduce(
                out=edge[:, 3:4], in_=xpad[:, 1:65, 64], axis=AX.X, op=ADD)
            # stt chain: mxs = sum_i coef_i * term_i  (coefs carry sign+1/HW)
            terms = [edge[:, 0:1], edge[:, 1:2], edge[:, 2:3], edge[:, 3:4],
                     xpad[:, 1, 1:2], xpad[:, 1, 64:65],
                     xpad[:, 64, 1:2], xpad[:, 64, 64:65]]
            nc.vector.tensor_scalar(
                out=accs[:, 0:1], in0=sx, scalar1=t_mc[:, 0:1], scalar2=None,
                op0=MULT)
            for i, term in enumerate(terms):
                dst = accs[:, i + 1:i + 2] if i < 7 else mxs
                nc.vector.scalar_tensor_tensor(
                    out=dst, in0=term, scalar=t_mc[:, i + 1:i + 2],
                    in1=accs[:, i:i + 1], op0=MULT, op1=ADD)

            # ---------- phase A: Xs (static depthwise) + sf --------------
            # PE program order is the execution order (in-order SEQ), so:
            # Xs0, Xs1, sf0, Xs2, sf1, [ctx], Xs3, sf2, sf3 - each sf(t)
            # waits on the ACT copy of xs(t), which runs under Xs(t+1).
            # The ctx branch slots into the tail where PE has slack; its
            # means inputs are DVE-computed straight from X2/Y2 above.
            def emit_xs(t, psA):
                xs_ps = psA.tile([C, 2, 512], F32, tag="xs_ps")
                for h in range(2):
                    for k in range(NT):
                        dh, dw = divmod(k, 3)
                        r0 = 16 * t + 8 * h + dh
                        rhs = xpad[:, r0:r0 + 8, dw:dw + W]
                        nc.tensor.matmul(
                            xs_ps[:, h, :],
                            t_dsw[:, k * C:(k + 1) * C],
                            rhs,
                            start=(k == 0),
                            stop=(k == NT - 1),
                        )
                nc.scalar.copy(out=xs[:, t * STN:(t + 1) * STN], in_=xs_ps)

            def emit_sf(t, psSF):
                sf_ps = psSF.tile([MREP, 2, 512], F32, tag="sf_ps")
                _absorb(nc, xs[0:1, t * STN:t * STN + 1],
                        sf_ps[0:1, 0, 0:1])
                for h in range(2):
                    c0 = t * STN + h * 512
                    nc.tensor.matmul(
                        sf_ps[:, h, :], t_wsa, xs[:, c0:c0 + 512],
                        start=True, stop=False)
                    nc.tensor.matmul(
                        sf_ps[:, h, :], t_wsb, y2[:, c0:c0 + 512],
                        start=False, stop=True)
                nc.scalar.copy(
                    out=sfs[:, t * STN:(t + 1) * STN], in_=sf_ps)

            def emit_ctx(psX):
                ctx_ps = psX.tile([C, 11], F32, tag="ctx")
                ctx1_ps = ctx_ps[0:64, 0:1]
                ctx2_ps = ctx_ps[0:64, 1:2]
                cf_ps = ctx_ps[:, 2:11]
                _absorb(nc, mxs, ctx1_ps[0:1, 0:1])
                nc.tensor.matmul(ctx1_ps, t_w1a, mxs, start=True, stop=False)
                nc.tensor.matmul(ctx1_ps, t_w1b, my2, start=False, stop=True)
                nc.scalar.copy(out=ctx1, in_=ctx1_ps)

                nc.tensor.matmul(ctx2_ps, t_w2t, ctx1, start=True, stop=True)
                nc.scalar.activation(out=ctx2, in_=ctx2_ps, func=ACT_RELU)

                for k in range(NT):
                    nc.tensor.matmul(
                        cf_ps[:, k:k + 1], t_w3t[:, k * C:(k + 1) * C],
                        ctx2, start=True, stop=True)
                nc.scalar.copy(out=cfsb, in_=cf_ps)

            with tc.tile_pool(name="psA", bufs=2, space="PSUM") as psA, \
                 tc.tile_pool(name="psSF", bufs=1, space="PSUM") as psSF, \
                 tc.tile_pool(name="psCtx", bufs=1, space="PSUM") as psX:
                emit_xs(0, psA)
                emit_xs(1, psA)
                emit_sf(0, psSF)
                emit_xs(2, psA)
                emit_sf(1, psSF)
                emit_ctx(psX)
                emit_xs(3, psA)
                emit_sf(2, psSF)
                emit_sf(3, psSF)

            # ---------- phase C: dynamic filter + fusion conv ------------
            # Per tile, the PE stream is pipelined for lookahead: the three
            # Pool taps' broadcasts + ACT bias-copies are issued FIRST (their
            # producer chain is ~3us deep), their fusion matmuls run LAST;
            # the six DVE taps run in the middle with bc one tap ahead of
            # the fusion matmul. PSUM: psBC 2x2 + psO 2x2 = 8 banks.
            # 8 half-tiles of 512 px: 1 PSUM bank per bc / out tile gives a
            # deep psBC lookahead (bufs=5) so DVE/Pool producers never wait
            # on PE emission. Pool taps interleave with DVE taps at the tile
            # head; their fusion matmuls rejoin mid-accumulation-chain.
            obf = ob.rearrange("c h w -> c (h w)")
            with tc.tile_pool(name="psBC", bufs=5, space="PSUM") as psBC, \
                 tc.tile_pool(name="psOut", bufs=2, space="PSUM") as psO, \
                 tc.tile_pool(name="pP", bufs=8) as pP, \
                 tc.tile_pool(name="pDF", bufs=4) as pDF, \
                 tc.tile_pool(name="pOsb", bufs=2) as pOsb:
                DVE_TAPS = tuple(k for k in range(NT) if k not in ACT_TAPS)
                for u in range(2 * NST):
                    c0 = u * 512

                    def emit_bc(k, first=False):
                        g = k % 2
                        bc_ps = psBC.tile([C, 8, W], F32, tag="bc")
                        if first:
                            _absorb(nc, sfs[0:1, c0:c0 + 1],
                                    bc_ps[0:1, 0, 0:1])
                        nc.tensor.matmul(
                            bc_ps[:, :, :],
                            t_bct[32 * g:32 * g + NT, k * C:(k + 1) * C],
                            sfs[32 * g:32 * g + NT, c0:c0 + 512],
                            start=True, stop=True,
                            tile_position=(32 * g, 0),
                        )
                        return bc_ps

                    def xslice(k):
                        dh, dw = divmod(k, 3)
                        return xpad[:, 8 * u + dh:8 * u + dh + 8, dw:dw + W]

                    def emit_pool_tap(k, first=False):
                        bc_ps = emit_bc(k, first)
                        df = pDF.tile([C, 8, W], BF16, tag="df")
                        nc.scalar.activation(
                            out=df, in_=bc_ps, func=ACT_IDENT,
                            bias=cfsb[:, k:k + 1])
                        p_sb = pP.tile([C, 8, W], BF16, tag="p")
                        nc.gpsimd.tensor_tensor(
                            out=p_sb, in0=df, in1=xslice(k), op=MULT)
                        return p_sb

                    def emit_dve_tap(k):
                        bc_ps = emit_bc(k)
                        p_sb = pP.tile([C, 8, W], BF16, tag="p")
                        nc.vector.scalar_tensor_tensor(
                            out=p_sb, in0=bc_ps,
                            scalar=cfsb[:, k:k + 1], in1=xslice(k),
                            op0=ADD, op1=MULT)
                        return p_sb

                    def emit_wfb(p_sb, stop=False):
                        nc.tensor.matmul(
                            out_ps, t_wfb, p_sb,
                            start=False, stop=stop)

                    out_ps = psO.tile([C, 8, W], F32, tag="out_ps")
                    _absorb(nc, xs[0:1, c0:c0 + 1], out_ps[0:1, 0, 0:1])
                    nc.tensor.matmul(
                        out_ps, t_wfa, xs[:, c0:c0 + 512],
                        start=True, stop=False)
                    P = {}
                    P[ACT_TAPS[0]] = emit_pool_tap(ACT_TAPS[0], first=True)
                    P[DVE_TAPS[0]] = emit_dve_tap(DVE_TAPS[0])
                    P[ACT_TAPS[1]] = emit_pool_tap(ACT_TAPS[1])
                    P[DVE_TAPS[1]] = emit_dve_tap(DVE_TAPS[1])
                    emit_wfb(P[DVE_TAPS[0]])
                    P[ACT_TAPS[2]] = emit_pool_tap(ACT_TAPS[2])
                    P[DVE_TAPS[2]] = emit_dve_tap(DVE_TAPS[2])
                    emit_wfb(P[DVE_TAPS[1]])
                    P[DVE_TAPS[3]] = emit_dve_tap(DVE_TAPS[3])
                    emit_wfb(P[DVE_TAPS[2]])
                    emit_wfb(P[ACT_TAPS[0]])
                    P[DVE_TAPS[4]] = emit_dve_tap(DVE_TAPS[4])
                    emit_wfb(P[DVE_TAPS[3]])
                    emit_wfb(P[ACT_TAPS[1]])
                    P[DVE_TAPS[5]] = emit_dve_tap(DVE_TAPS[5])
                    emit_wfb(P[DVE_TAPS[4]])
                    emit_wfb(P[ACT_TAPS[2]])
                    emit_wfb(P[DVE_TAPS[5]], stop=True)
                    o_sb = pOsb.tile([C, 8, W], F32, tag="osb")
                    nc.scalar.copy(out=o_sb, in_=out_ps)
                    nc.sync.dma_start(
                        out=obf[:, c0:c0 + 512],
                        in_=o_sb.rearrange("c r w -> c (r w)"),
                    )
    _split_multiwaits(nc)
    return nc


def _prep_weights(static_w, w1, w2, w3, ws, wf):
    """Repack the tiny weights into the bf16 SBUF layouts."""
    f = np.float32
    sw = np.ascontiguousarray(static_w.reshape(C, NT), dtype=f)

    dsw = np.zeros((C, NT * C), dtype=f)
    for k in range(NT):
        dsw[np.arange(C), k * C + np.arange(C)] = sw[:, k]

    wsa = np.zeros((C, MREP), dtype=f)
    wsb = np.zeros((C, MREP), dtype=f)
    for g in range(4):
        for k in range(NT):
            wsa[:, 32 * g + k] = ws[k, :C]
            wsb[:, 32 * g + k] = ws[k, C:]

    w1a = np.ascontiguousarray(w1[:, :C].T, dtype=f)
    w1b = np.ascontiguousarray(w1[:, C:].T, dtype=f)
    w2t64 = np.ascontiguousarray(w2.T, dtype=f)
    w3t64 = np.ascontiguousarray(
        w3.reshape(C, NT, 64).transpose(2, 1, 0), dtype=f).reshape(64, NT * C)
    w2t = np.zeros((C, 64), dtype=f)
    w2t[:64] = w2t64
    w3t = np.zeros((C, NT * C), dtype=f)
    w3t[:64] = w3t64

    # mean-correction coefficients (sign and 1/HW folded in):
    # terms: [S, R0, R63, C0, C63, X00, X0_63, X63_0, X63_63]
    mc = np.zeros((C, 9), dtype=f)
    mc[:, 0] = sw.sum(axis=1)
    mc[:, 1] = -(sw[:, 6] + sw[:, 7] + sw[:, 8])   # dh=+1 excludes row0
    mc[:, 2] = -(sw[:, 0] + sw[:, 1] + sw[:, 2])   # dh=-1 excludes row63
    mc[:, 3] = -(sw[:, 2] + sw[:, 5] + sw[:, 8])   # dw=+1 excludes col0
    mc[:, 4] = -(sw[:, 0] + sw[:, 3] + sw[:, 6])   # dw=-1 excludes col63
    mc[:, 5] = sw[:, 8]
    mc[:, 6] = sw[:, 6]
    mc[:, 7] = sw[:, 2]
    mc[:, 8] = sw[:, 0]
    mc *= 1.0 / HW
    mc_bf = np.ascontiguousarray(mc).view(np.uint16).view(BFDT)  # [C, 18]

    wfa = np.ascontiguousarray(wf[:, :C].T, dtype=f)
    wfb = np.ascontiguousarray(wf[:, C:].T, dtype=f)

    bct = np.zeros((C, NT * C), dtype=f)
    for g in range(4):
        for k in range(NT):
            bct[32 * g + k, k * C:(k + 1) * C] = 1.0

    wts_pre = np.concatenate(
        [dsw, wsa, wsb, w1a, w1b, w2t, w3t], axis=1).astype(BFDT)
    wts_pre = np.concatenate([wts_pre, mc_bf], axis=1)
    wts_post = np.concatenate([wfa, wfb, bct], axis=1).astype(BFDT)
    return wts_pre, wts_post


def make_in_maps(X2, Y2, static_w, w1, w2, w3, ws, wf):
    wts_pre, wts_post = _prep_weights(
        np.asarray(static_w), np.asarray(w1), np.asarray(w2),
        np.asarray(w3), np.asarray(ws), np.asarray(wf),
    )
    X2 = np.asarray(X2)
    Y2 = np.asarray(Y2)
    xpad_all = np.zeros((B, C, PH, PW), dtype=np.float32)
    xpad_all[:, :, 1:H + 1, 1:W + 1] = X2
    xpad_all = xpad_all.reshape(B, C, PH * PW).astype(BFDT)
    y2_all = Y2.reshape(B, C, HW).astype(BFDT)
    in_maps = []
    for b in range(B):
        m = {"pk": np.ascontiguousarray(np.concatenate(
            [wts_pre, xpad_all[b], y2_all[b], wts_post], axis=1))}
        in_maps.append(m)
    return in_maps


def get_nc():
    if "nc" not in _CACHE:
        _CACHE["nc"] = _build_bass()
    return _CACHE["nc"]


def kernel(X2, Y2, static_w, w1, w2, w3, ws, wf):
    nc = get_nc()
    in_maps = make_in_maps(
        np.asarray(X2), np.asarray(Y2), static_w, w1, w2, w3, ws, wf
    )
    res = run_bass_kernel_spmd(nc, in_maps, core_ids=list(range(B)))
    out = np.stack([np.asarray(r["ob"], dtype=np.float32)
                    for r in res.results])
    return out


# revision 16
# speedup vs baseline: 1.6743x; 1.0986x over previous
"""Trainium2 Bass kernel for the CMDF block (dense_cnn), V3.

Contract: kernel(**inputs) takes the FULL unsharded inputs (B=8, C=128,
H=W=64) and returns the FULL (8, 128, 64, 64) float32 output.

Sharding: data-parallel over batch - core b computes batch element b.
All weights are replicated (host-side prepacked into matmul layouts).

Math per batch element (see reference):
  Xs   = depthwise3x3(X2, static_w)
  ctx  = relu(w2 @ (w1 @ mean_hw([Xs; Y2])))
  cf   = (w3 @ ctx).reshape(C, 9)          # per-channel dynamic filter
  sf   = ws @ [Xs; Y2]                     # (9, H, W) spatial filter
  dyn  = sum_k shift_k(X2) * (cf[:, k] + sf[k])
  out  = wf[:, :C] @ Xs + wf[:, C:] @ dyn

V3 design notes (vs the f32r V1):
  - Everything streams in bf16 (3.2MB input vs 6.4MB): DMA lead-in halves.
    Weights land first (chunked DMA) so PE starts at ~3us, not ~21us.
  - mean_hw(Xs) is computed WITHOUT Xs: mean of a zero-padded shifted
    image = S - edge strips + corners, so a per-tap host-precomputed
    coefficient vector turns 9 shifted means into one stt chain over
    {S, row0, row63, col0, col63, 4 corners}. The context branch then
    runs concurrently with phase A instead of serializing after it.
  - Phase C taps are split across engines: 6 taps as DVE stt reading the
    broadcast filter from PSUM; 3 taps via ACT Identity-with-bias copy
    (PSUM -> SBUF bf16, folds cf in) followed by a Pool tensor_tensor
    multiply. PE does the 9 broadcast + 9 fusion matmuls per tile and is
    the ~8.1us/tile bottleneck; DVE/ACT/Pool hide underneath.
"""

import numpy as np
import ml_dtypes

import concourse.bass as bass
import concourse.tile as tile
import concourse.mybir as mybir
from concourse.bass_utils import run_bass_kernel_spmd

B, C, H, W, K = 8, 128, 64, 64, 3
HW = H * W            # 4096
PH, PW = H + 2, W + 2  # 66, 66 padded
NST = 4               # super-tiles over rows
ROWS = H // NST       # 16 image rows per super-tile
STN = ROWS * W        # 1024 pixels per super-tile (2 PSUM banks)
NT = K * K            # 9 taps
MREP = 3 * 32 + NT    # 105: ws replicated at partition groups 0,32,64,96

F32 = mybir.dt.float32
BF16 = mybir.dt.bfloat16
FP8 = mybir.dt.float8e4
DROW = mybir.MatmulPerfMode.DoubleRow
ADD = mybir.AluOpType.add
MULT = mybir.AluOpType.mult
AX = mybir.AxisListType
ACT_COPY = mybir.ActivationFunctionType.Copy
ACT_IDENT = mybir.ActivationFunctionType.Identity
ACT_RELU = mybir.ActivationFunctionType.Relu

BFDT = ml_dtypes.bfloat16
F8DT = ml_dtypes.float8_e4m3fn

# ---- pk8 (fp8) column layout: DoubleRow channel-paired, 64 partitions ----
O8_DSW = 0                      # [64, 2, NT*C] depthwise diag
O8_XP = 2 * NT * C              # 2304; [64, 2, PH*PW] padded X2
PK8_COLS = O8_XP + 2 * PH * PW  # 11016

# taps handled by ACT(bias)+Pool(mult); the rest are DVE stt
ACT_TAPS = (0, 1, 2)

# ---- pk column layout (bf16 columns) ----
O_WSA = 0
O_WSB = O_WSA + MREP          # 105
O_W1A = O_WSB + MREP          # 210
O_W1B = O_W1A + 64
O_W2T = O_W1B + 64
O_W3T = O_W2T + 64
O_MC = O_W3T + NT * C         # (9 f32 coeffs = 18 bf16 cols)
O_XP = O_MC + 18
O_Y2 = O_XP + PH * PW
O_WFA = O_Y2 + HW
O_WFB = O_WFA + C
O_BCT = O_WFB + C
PK_COLS = O_BCT + NT * C

_CACHE = {}


def _absorb(nc, dep_elem, ps_elem):
    """Tiny bf16 matmul that reads one element of `dep_elem` and writes a
    junk element of `ps_elem` (later overwritten by a start=True group).
    Purpose: acquire the semaphore wait on dep_elem's producer on a plain
    (non-fused) matmul, so the following fused matmul - which can embed
    only ONE sem wait - doesn't need two."""
    lh = dep_elem
    nc.tensor.matmul(ps_elem, lh[:, 0:1], lh[:, 0:1], start=True, stop=True)


def _split_multiwaits(nc):
    """walrus codegen in this toolchain accepts only ONE embedded sem wait
    per instruction. Hoist excess waits onto same-engine NoOps placed
    immediately before the instruction (engines execute in order, so the
    blocking behavior is identical)."""
    ctr = 0
    for fn in nc.m.functions:
        for blk in fn.blocks:
            insts = blk.instructions
            out = []
            for inst in insts:
                si = inst.sync_info
                waits = list(si.on_wait) if si is not None and si.on_wait else []
                if len(waits) > 1:
                    for w in waits[:-1]:
                        ctr += 1
                        out.append(mybir.InstNoOp(
                            name=f"I-wsplit-{ctr}",
                            engine=inst.engine,
                            ins=[], outs=[],
                            sync_info=mybir.SyncInfo(
                                on_wait=[w], on_update=[]),
                        ))
                    inst.sync_info = mybir.SyncInfo(
                        on_wait=[waits[-1]],
                        on_update=list(si.on_update) if si.on_update else [],
                    )
                out.append(inst)
            blk.instructions = out


def _build_bass():
    nc = bass.Bass("TRN2", target_bir_lowering=False, debug=False)

    pk = nc.dram_tensor("pk", [C, PK_COLS], BF16, kind="ExternalInput").ap()
    pk8 = nc.dram_tensor("pk8", [64, PK8_COLS], FP8, kind="ExternalInput").ap()
    ob = nc.dram_tensor("ob", [C, H, W], F32, kind="ExternalOutput").ap()

    with tile.TileContext(nc) as tc:
        with tc.tile_pool(name="singles", bufs=1) as S:
            stg = S.tile([C, PK_COLS], BF16)
            stg8 = S.tile([64, PK8_COLS], FP8)
            t_dsw8 = stg8[:, O8_DSW:O8_DSW + 2 * NT * C].rearrange(
                "p (i m) -> p i m", i=2)
            xpad8 = stg8[:, O8_XP:O8_XP + 2 * PH * PW].rearrange(
                "p (i h w) -> p i h w", i=2, w=PW)
            t_wsa = stg[:, O_WSA:O_WSA + MREP]
            t_wsb = stg[:, O_WSB:O_WSB + MREP]
            t_w1a = stg[:, O_W1A:O_W1A + 64]
            t_w1b = stg[:, O_W1B:O_W1B + 64]
            t_w2t = stg[0:64, O_W2T:O_W2T + 64]
            t_w3t = stg[0:64, O_W3T:O_W3T + NT * C]
            t_mc = stg[:, O_MC:O_MC + 18].bitcast(F32)  # [C, 9] f32
            xpad = stg[:, O_XP:O_XP + PH * PW].rearrange(
                "p (h w) -> p h w", w=PW)
            xpflat = stg[:, O_XP:O_XP + PH * PW]
            y2 = stg[:, O_Y2:O_Y2 + HW]
            t_wfa = stg[:, O_WFA:O_WFA + C]
            t_wfb = stg[:, O_WFB:O_WFB + C]
            t_bct = stg[:, O_BCT:O_BCT + NT * C]

            xs = S.tile([C, HW], BF16)
            sfs = S.tile([MREP, HW], BF16)

            xpart = S.tile([C, 4], F32)
            y2part = S.tile([C, 2], F32)
            sx = S.tile([C, 1], F32)
            y2sum = S.tile([C, 1], F32)
            edge = S.tile([C, 4], F32)   # R0, R63, C0, C63
            accs = S.tile([C, 8], F32)
            mxs = S.tile([C, 1], BF16)
            my2 = S.tile([C, 1], BF16)
            ctx1 = S.tile([64, 1], BF16)
            ctx2 = S.tile([64, 1], BF16)
            cfsb = S.tile([C, NT], F32)

            # ---------- input DMAs: fp8 depthwise operands first ---------
            XPB = [0, 18 * PW, 34 * PW, 50 * PW, PH * PW]
            nc.sync.dma_start(out=stg8[:, 0:O8_XP], in_=pk8[:, 0:O8_XP])
            for i in range(2):
                a = O8_XP + i * PH * PW + XPB[0]
                b = O8_XP + i * PH * PW + XPB[1]
                nc.sync.dma_start(out=stg8[:, a:b], in_=pk8[:, a:b])
            nc.sync.dma_start(out=stg[:, 0:O_W1A], in_=pk[:, 0:O_W1A])
            for j in range(1, 4):
                for i in range(2):
                    a = O8_XP + i * PH * PW + XPB[j]
                    b = O8_XP + i * PH * PW + XPB[j + 1]
                    nc.sync.dma_start(out=stg8[:, a:b], in_=pk8[:, a:b])
            for i in range(2):
                a, b = O_Y2 + 2048 * i, O_Y2 + 2048 * (i + 1)
                nc.sync.dma_start(out=stg[:, a:b], in_=pk[:, a:b])
            for i in range(4):
                a, b = O_XP + XPB[i], O_XP + XPB[i + 1]
                nc.sync.dma_start(out=stg[:, a:b], in_=pk[:, a:b])
            nc.sync.dma_start(out=stg[:, O_W1A:O_XP], in_=pk[:, O_W1A:O_XP])
            nc.sync.dma_start(out=stg[:, O_WFA:], in_=pk[:, O_WFA:])

            # ---------- means from X2/Y2 directly (DVE, overlaps A) ------
            for i in range(4):
                nc.vector.tensor_reduce(
                    out=xpart[:, i:i + 1],
                    in_=xpflat[:, XPB[i]:XPB[i + 1]], axis=AX.X, op=ADD)
            nc.vector.tensor_reduce(out=sx, in_=xpart, axis=AX.X, op=ADD)
            for i in range(2):
                nc.vector.tensor_reduce(
                    out=y2part[:, i:i + 1],
                    in_=y2[:, 2048 * i:2048 * (i + 1)], axis=AX.X, op=ADD)
            nc.vector.tensor_reduce(out=y2sum, in_=y2part, axis=AX.X, op=ADD)
            nc.vector.tensor_scalar(
                out=my2, in0=y2sum, scalar1=1.0 / HW, scalar2=None, op0=MULT)
            # edge strips (row 0/63, col 0/63 of the unpadded image)
            nc.vector.tensor_reduce(
                out=edge[:, 0:1], in_=xpad[:, 1, 1:65], axis=AX.X, op=ADD)
            nc.vector.tensor_reduce(
                out=edge[:, 1:2], in_=xpad[:, 64, 1:65], axis=AX.X, op=ADD)
            nc.vector.tensor_reduce(
                out=edge[:, 2:3], in_=xpad[:, 1:65, 1], axis=AX.X, op=ADD)
            nc.vector.tensor_reduce(
                out=edge[:, 3:4], in_=xpad[:, 1:65, 64], axis=AX.X, op=ADD)
            # stt chain: mxs = sum_i coef_i * term_i  (coefs carry sign+1/HW)
            terms = [edge[:, 0:1], edge[:, 1:2], edge[:, 2:3], edge[:, 3:4],
                     xpad[:, 1, 1:2], xpad[:, 1, 64:65],
                     xpad[:, 64, 1:2], xpad[:, 64, 64:65]]
            nc.vector.tensor_scalar(
                out=accs[:, 0:1], in0=sx, scalar1=t_mc[:, 0:1], scalar2=None,
                op0=MULT)
            for i, term in enumerate(terms):
                dst = accs[:, i + 1:i + 2] if i < 7 else mxs
                nc.vector.scalar_tensor_tensor(
                    out=dst, in0=term, scalar=t_mc[:, i + 1:i + 2],
                    in1=accs[:, i:i + 1], op0=MULT, op1=ADD)

            # ---------- phase A: Xs (static depthwise) + sf --------------
            # PE program order is the execution order (in-order SEQ), so:
            # Xs0, Xs1, sf0, Xs2, sf1, [ctx], Xs3, sf2, sf3 - each sf(t)
            # waits on the ACT copy of xs(t), which runs under Xs(t+1).
            # The ctx branch slots into the tail where PE has slack; its
            # means inputs are DVE-computed straight from X2/Y2 above.
            def emit_xs(t, psA):
                xs_ps = psA.tile([C, 2, 512], F32, tag="xs_ps")
                for h in range(2):
                    for k in range(NT):
                        dh, dw = divmod(k, 3)
                        r0 = 16 * t + 8 * h + dh
                        rhs = xpad8[:, :, r0:r0 + 8, dw:dw + W]
                        nc.tensor.matmul(
                            xs_ps[:, h, :],
                            t_dsw8[:, :, k * C:(k + 1) * C],
                            rhs,
                            start=(k == 0),
                            stop=(k == NT - 1),
                            perf_mode=DROW,
                        )
                nc.scalar.copy(out=xs[:, t * STN:(t + 1) * STN], in_=xs_ps)

            def emit_sf(t, psSF):
                sf_ps = psSF.tile([MREP, 2, 512], F32, tag="sf_ps")
                _absorb(nc, xs[0:1, t * STN:t * STN + 1],
                        sf_ps[0:1, 0, 0:1])
                for h in range(2):
                    c0 = t * STN + h * 512
                    nc.tensor.matmul(
                        sf_ps[:, h, :], t_wsa, xs[:, c0:c0 + 512],
                        start=True, stop=False)
                    nc.tensor.matmul(
                        sf_ps[:, h, :], t_wsb, y2[:, c0:c0 + 512],
                        start=False, stop=True)
                nc.scalar.copy(
                    out=sfs[:, t * STN:(t + 1) * STN], in_=sf_ps)

            def emit_ctx(psX):
                ctx_ps = psX.tile([C, 11], F32, tag="ctx")
                ctx1_ps = ctx_ps[0:64, 0:1]
                ctx2_ps = ctx_ps[0:64, 1:2]
                cf_ps = ctx_ps[:, 2:11]
                _absorb(nc, mxs, ctx1_ps[0:1, 0:1])
                nc.tensor.matmul(ctx1_ps, t_w1a, mxs, start=True, stop=False)
                nc.tensor.matmul(ctx1_ps, t_w1b, my2, start=False, stop=True)
                nc.scalar.copy(out=ctx1, in_=ctx1_ps)

                nc.tensor.matmul(ctx2_ps, t_w2t, ctx1, start=True, stop=True)
                nc.scalar.activation(out=ctx2, in_=ctx2_ps, func=ACT_RELU)

                for k in range(NT):
                    nc.tensor.matmul(
                        cf_ps[:, k:k + 1], t_w3t[:, k * C:(k + 1) * C],
                        ctx2, start=True, stop=True)
                nc.scalar.copy(out=cfsb, in_=cf_ps)

            with tc.tile_pool(name="psA", bufs=2, space="PSUM") as psA, \
                 tc.tile_pool(name="psSF", bufs=1, space="PSUM") as psSF, \
                 tc.tile_pool(name="psCtx", bufs=1, space="PSUM") as psX:
                emit_xs(0, psA)
                emit_xs(1, psA)
                emit_sf(0, psSF)
                emit_xs(2, psA)
                emit_sf(1, psSF)
                emit_ctx(psX)
                emit_xs(3, psA)
                emit_sf(2, psSF)
                emit_sf(3, psSF)

            # ---------- phase C: dynamic filter + fusion conv ------------
            # Per tile, the PE stream is pipelined for lookahead: the three
            # Pool taps' broadcasts + ACT bias-copies are issued FIRST (their
            # producer chain is ~3us deep), their fusion matmuls run LAST;
            # the six DVE taps run in the middle with bc one tap ahead of
            # the fusion matmul. PSUM: psBC 2x2 + psO 2x2 = 8 banks.
            # 8 half-tiles of 512 px: 1 PSUM bank per bc / out tile gives a
            # deep psBC lookahead (bufs=5) so DVE/Pool producers never wait
            # on PE emission. Pool taps interleave with DVE taps at the tile
            # head; their fusion matmuls rejoin mid-accumulation-chain.
            obf = ob.rearrange("c h w -> c (h w)")
            with tc.tile_pool(name="psBC", bufs=5, space="PSUM") as psBC, \
                 tc.tile_pool(name="psOut", bufs=2, space="PSUM") as psO, \
                 tc.tile_pool(name="pP", bufs=8) as pP, \
                 tc.tile_pool(name="pDF", bufs=4) as pDF, \
                 tc.tile_pool(name="pOsb", bufs=2) as pOsb:
                DVE_TAPS = tuple(k for k in range(NT) if k not in ACT_TAPS)
                for u in range(2 * NST):
                    c0 = u * 512

                    def emit_bc(k, first=False):
                        g = k % 2
                        bc_ps = psBC.tile([C, 8, W], F32, tag="bc")
                        if first:
                            _absorb(nc, sfs[0:1, c0:c0 + 1],
                                    bc_ps[0:1, 0, 0:1])
                        nc.tensor.matmul(
                            bc_ps[:, :, :],
                            t_bct[32 * g:32 * g + NT, k * C:(k + 1) * C],
                            sfs[32 * g:32 * g + NT, c0:c0 + 512],
                            start=True, stop=True,
                            tile_position=(32 * g, 0),
                        )
                        return bc_ps

                    def xslice(k):
                        dh, dw = divmod(k, 3)
                        return xpad[:, 8 * u + dh:8 * u + dh + 8, dw:dw + W]

                    def emit_pool_tap(k, first=False):
                        bc_ps = emit_bc(k, first)
                        df = pDF.tile([C, 8, W], BF16, tag="df")
                        nc.scalar.activation(
                            out=df, in_=bc_ps, func=ACT_IDENT,
                            bias=cfsb[:, k:k + 1])
                        p_sb = pP.tile([C, 8, W], BF16, tag="p")
                        nc.gpsimd.tensor_tensor(
                            out=p_sb, in0=df, in1=xslice(k), op=MULT)
                        return p_sb

                    def emit_dve_tap(k):
                        bc_ps = emit_bc(k)
                        p_sb = pP.tile([C, 8, W], BF16, tag="p")
                        nc.vector.scalar_tensor_tensor(
                            out=p_sb, in0=bc_ps,
                            scalar=cfsb[:, k:k + 1], in1=xslice(k),
                            op0=ADD, op1=MULT)
                        return p_sb

                    def emit_wfb(p_sb, stop=False):
                        nc.tensor.matmul(
                            out_ps, t_wfb, p_sb,
                            start=False, stop=stop)

                    out_ps = psO.tile([C, 8, W], F32, tag="out_ps")
                    _absorb(nc, xs[0:1, c0:c0 + 1], out_ps[0:1, 0, 0:1])
                    nc.tensor.matmul(
                        out_ps, t_wfa, xs[:, c0:c0 + 512],
                        start=True, stop=False)
                    P = {}
                    P[ACT_TAPS[0]] = emit_pool_tap(ACT_TAPS[0], first=True)
                    P[DVE_TAPS[0]] = emit_dve_tap(DVE_TAPS[0])
                    P[ACT_TAPS[1]] = emit_pool_tap(ACT_TAPS[1])
                    P[DVE_TAPS[1]] = emit_dve_tap(DVE_TAPS[1])
                    emit_wfb(P[DVE_TAPS[0]])
                    P[ACT_TAPS[2]] = emit_pool_tap(ACT_TAPS[2])
                    P[DVE_TAPS[2]] = emit_dve_tap(DVE_TAPS[2])
                    emit_wfb(P[DVE_TAPS[1]])
                    P[DVE_TAPS[3]] = emit_dve_tap(DVE_TAPS[3])
                    emit_wfb(P[DVE_TAPS[2]])
                    emit_wfb(P[ACT_TAPS[0]])
                    P[DVE_TAPS[4]] = emit_dve_tap(DVE_TAPS[4])
                    emit_wfb(P[DVE_TAPS[3]])
                    emit_wfb(P[ACT_TAPS[1]])
                    P[DVE_TAPS[5]] = emit_dve_tap(DVE_TAPS[5])
                    emit_wfb(P[DVE_TAPS[4]])
                    emit_wfb(P[ACT_TAPS[2]])
                    emit_wfb(P[DVE_TAPS[5]], stop=True)
                    o_sb = pOsb.tile([C, 8, W], F32, tag="osb")
                    nc.scalar.copy(out=o_sb, in_=out_ps)
                    nc.sync.dma_start(
                        out=obf[:, c0:c0 + 512],
                        in_=o_sb.rearrange("c r w -> c (r w)"),
                    )
    _split_multiwaits(nc)
    return nc


def _prep_weights(static_w, w1, w2, w3, ws, wf):
    """Repack the tiny weights into the bf16 SBUF layouts."""
    f = np.float32
    sw = np.ascontiguousarray(static_w.reshape(C, NT), dtype=f)

    # fp8 DoubleRow depthwise weights: [64, 2, NT*C]
    dsw8 = np.zeros((64, 2, NT * C), dtype=f)
    for k in range(NT):
        for m in range(C):
            dsw8[m % 64, m // 64, k * C + m] = sw[m, k]
    dsw8 = dsw8.reshape(64, 2 * NT * C).astype(F8DT)

    wsa = np.zeros((C, MREP), dtype=f)
    wsb = np.zeros((C, MREP), dtype=f)
    for g in range(4):
        for k in range(NT):
            wsa[:, 32 * g + k] = ws[k, :C]
            wsb[:, 32 * g + k] = ws[k, C:]

    w1a = np.ascontiguousarray(w1[:, :C].T, dtype=f)
    w1b = np.ascontiguousarray(w1[:, C:].T, dtype=f)
    w2t64 = np.ascontiguousarray(w2.T, dtype=f)
    w3t64 = np.ascontiguousarray(
        w3.reshape(C, NT, 64).transpose(2, 1, 0), dtype=f).reshape(64, NT * C)
    w2t = np.zeros((C, 64), dtype=f)
    w2t[:64] = w2t64
    w3t = np.zeros((C, NT * C), dtype=f)
    w3t[:64] = w3t64

    # mean-correction coefficients (sign and 1/HW folded in):
    # terms: [S, R0, R63, C0, C63, X00, X0_63, X63_0, X63_63]
    mc = np.zeros((C, 9), dtype=f)
    mc[:, 0] = sw.sum(axis=1)
    mc[:, 1] = -(sw[:, 6] + sw[:, 7] + sw[:, 8])   # dh=+1 excludes row0
    mc[:, 2] = -(sw[:, 0] + sw[:, 1] + sw[:, 2])   # dh=-1 excludes row63
    mc[:, 3] = -(sw[:, 2] + sw[:, 5] + sw[:, 8])   # dw=+1 excludes col0
    mc[:, 4] = -(sw[:, 0] + sw[:, 3] + sw[:, 6])   # dw=-1 excludes col63
    mc[:, 5] = sw[:, 8]
    mc[:, 6] = sw[:, 6]
    mc[:, 7] = sw[:, 2]
    mc[:, 8] = sw[:, 0]
    mc *= 1.0 / HW
    mc_bf = np.ascontiguousarray(mc).view(np.uint16).view(BFDT)  # [C, 18]

    wfa = np.ascontiguousarray(wf[:, :C].T, dtype=f)
    wfb = np.ascontiguousarray(wf[:, C:].T, dtype=f)

    bct = np.zeros((C, NT * C), dtype=f)
    for g in range(4):
        for k in range(NT):
            bct[32 * g + k, k * C:(k + 1) * C] = 1.0

    wts_pre = np.concatenate(
        [wsa, wsb, w1a, w1b, w2t, w3t], axis=1).astype(BFDT)
    wts_pre = np.concatenate([wts_pre, mc_bf], axis=1)
    wts_post = np.concatenate([wfa, wfb, bct], axis=1).astype(BFDT)
    return wts_pre, wts_post, dsw8


def make_in_maps(X2, Y2, static_w, w1, w2, w3, ws, wf):
    wts_pre, wts_post, dsw8 = _prep_weights(
        np.asarray(static_w), np.asarray(w1), np.asarray(w2),
        np.asarray(w3), np.asarray(ws), np.asarray(wf),
    )
    X2 = np.asarray(X2)
    Y2 = np.asarray(Y2)
    xpad_f = np.zeros((B, C, PH, PW), dtype=np.float32)
    xpad_f[:, :, 1:H + 1, 1:W + 1] = X2
    xpad_all = xpad_f.reshape(B, C, PH * PW).astype(BFDT)
    xp8_all = xpad_f.reshape(B, 2, 64, PH * PW).transpose(0, 2, 1, 3).reshape(
        B, 64, 2 * PH * PW).astype(F8DT)
    y2_all = Y2.reshape(B, C, HW).astype(BFDT)
    in_maps = []
    for b in range(B):
        m = {"pk": np.ascontiguousarray(np.concatenate(
                [wts_pre, xpad_all[b], y2_all[b], wts_post], axis=1)),
             "pk8": np.ascontiguousarray(np.concatenate(
                [dsw8, xp8_all[b]], axis=1))}
        in_maps.append(m)
    return in_maps


def get_nc():
    if "nc" not in _CACHE:
        _CACHE["nc"] = _build_bass()
    return _CACHE["nc"]


def kernel(X2, Y2, static_w, w1, w2, w3, ws, wf):
    nc = get_nc()
    in_maps = make_in_maps(
        np.asarray(X2), np.asarray(Y2), static_w, w1, w2, w3, ws, wf
    )
    res = run_bass_kernel_spmd(nc, in_maps, core_ids=list(range(B)))
    out = np.stack([np.asarray(r["ob"], dtype=np.float32)
                    for r in res.results])
    return out


# revision 17
# speedup vs baseline: 1.7299x; 1.0332x over previous
"""Trainium2 Bass kernel for the CMDF block (dense_cnn), V3.

Contract: kernel(**inputs) takes the FULL unsharded inputs (B=8, C=128,
H=W=64) and returns the FULL (8, 128, 64, 64) float32 output.

Sharding: data-parallel over batch - core b computes batch element b.
All weights are replicated (host-side prepacked into matmul layouts).

Math per batch element (see reference):
  Xs   = depthwise3x3(X2, static_w)
  ctx  = relu(w2 @ (w1 @ mean_hw([Xs; Y2])))
  cf   = (w3 @ ctx).reshape(C, 9)          # per-channel dynamic filter
  sf   = ws @ [Xs; Y2]                     # (9, H, W) spatial filter
  dyn  = sum_k shift_k(X2) * (cf[:, k] + sf[k])
  out  = wf[:, :C] @ Xs + wf[:, C:] @ dyn

V3 design notes (vs the f32r V1):
  - Everything streams in bf16 (3.2MB input vs 6.4MB): DMA lead-in halves.
    Weights land first (chunked DMA) so PE starts at ~3us, not ~21us.
  - mean_hw(Xs) is computed WITHOUT Xs: mean of a zero-padded shifted
    image = S - edge strips + corners, so a per-tap host-precomputed
    coefficient vector turns 9 shifted means into one stt chain over
    {S, row0, row63, col0, col63, 4 corners}. The context branch then
    runs concurrently with phase A instead of serializing after it.
  - Phase C taps are split across engines: 6 taps as DVE stt reading the
    broadcast filter from PSUM; 3 taps via ACT Identity-with-bias copy
    (PSUM -> SBUF bf16, folds cf in) followed by a Pool tensor_tensor
    multiply. PE does the 9 broadcast + 9 fusion matmuls per tile and is
    the ~8.1us/tile bottleneck; DVE/ACT/Pool hide underneath.
"""

import numpy as np
import ml_dtypes

import concourse.bass as bass
import concourse.tile as tile
import concourse.mybir as mybir
from concourse.bass_utils import run_bass_kernel_spmd

B, C, H, W, K = 8, 128, 64, 64, 3
HW = H * W            # 4096
PH, PW = H + 2, W + 2  # 66, 66 padded
NST = 4               # super-tiles over rows
ROWS = H // NST       # 16 image rows per super-tile
STN = ROWS * W        # 1024 pixels per super-tile (2 PSUM banks)
NT = K * K            # 9 taps
MREP = 3 * 32 + NT    # 105: ws replicated at partition groups 0,32,64,96

F32 = mybir.dt.float32
BF16 = mybir.dt.bfloat16
FP8 = mybir.dt.float8e4
DROW = mybir.MatmulPerfMode.DoubleRow
ADD = mybir.AluOpType.add
MULT = mybir.AluOpType.mult
AX = mybir.AxisListType
ACT_COPY = mybir.ActivationFunctionType.Copy
ACT_IDENT = mybir.ActivationFunctionType.Identity
ACT_RELU = mybir.ActivationFunctionType.Relu

BFDT = ml_dtypes.bfloat16
F8DT = ml_dtypes.float8_e4m3fn

# ---- pk8 (fp8) column layout: DoubleRow channel-paired, 64 partitions ----
O8_DSW = 0                      # [64, 2, NT*C] depthwise diag
O8_XP = 2 * NT * C              # 2304; [64, PH, 2, PW] padded X2 (row-major)
PK8_COLS = O8_XP + 2 * PH * PW  # 11016

# taps handled by ACT(bias)+Pool(mult); the rest are DVE stt
ACT_TAPS = (0, 1, 2)

# ---- pk column layout (bf16 columns) ----
O_WSA = 0
O_WSB = O_WSA + MREP          # 105
O_Y2 = O_WSB + MREP           # 210
O_XP = O_Y2 + HW
O_W1A = O_XP + PH * PW
O_W1B = O_W1A + 64
O_W2T = O_W1B + 64
O_W3T = O_W2T + 64
O_MC = O_W3T + NT * C         # (9 f32 coeffs = 18 bf16 cols)
O_WFA = O_MC + 18
O_WFB = O_WFA + C
O_BCT = O_WFB + C
PK_COLS = O_BCT + NT * C

_CACHE = {}


def _absorb(nc, dep_elem, ps_elem):
    """Tiny bf16 matmul that reads one element of `dep_elem` and writes a
    junk element of `ps_elem` (later overwritten by a start=True group).
    Purpose: acquire the semaphore wait on dep_elem's producer on a plain
    (non-fused) matmul, so the following fused matmul - which can embed
    only ONE sem wait - doesn't need two."""
    lh = dep_elem
    nc.tensor.matmul(ps_elem, lh[:, 0:1], lh[:, 0:1], start=True, stop=True)


def _split_multiwaits(nc):
    """walrus codegen in this toolchain accepts only ONE embedded sem wait
    per instruction. Hoist excess waits onto same-engine NoOps placed
    immediately before the instruction (engines execute in order, so the
    blocking behavior is identical)."""
    ctr = 0
    for fn in nc.m.functions:
        for blk in fn.blocks:
            insts = blk.instructions
            out = []
            for inst in insts:
                si = inst.sync_info
                waits = list(si.on_wait) if si is not None and si.on_wait else []
                if len(waits) > 1:
                    for w in waits[:-1]:
                        ctr += 1
                        out.append(mybir.InstNoOp(
                            name=f"I-wsplit-{ctr}",
                            engine=inst.engine,
                            ins=[], outs=[],
                            sync_info=mybir.SyncInfo(
                                on_wait=[w], on_update=[]),
                        ))
                    inst.sync_info = mybir.SyncInfo(
                        on_wait=[waits[-1]],
                        on_update=list(si.on_update) if si.on_update else [],
                    )
                out.append(inst)
            blk.instructions = out


def _build_bass():
    nc = bass.Bass("TRN2", target_bir_lowering=False, debug=False)

    pk = nc.dram_tensor("pk", [C, PK_COLS], BF16, kind="ExternalInput").ap()
    pk8 = nc.dram_tensor("pk8", [64, PK8_COLS], FP8, kind="ExternalInput").ap()
    ob = nc.dram_tensor("ob", [C, H, W], F32, kind="ExternalOutput").ap()

    with tile.TileContext(nc) as tc:
        with tc.tile_pool(name="singles", bufs=1) as S:
            stg = S.tile([C, PK_COLS], BF16)
            stg8 = S.tile([64, PK8_COLS], FP8)
            t_dsw8 = stg8[:, O8_DSW:O8_DSW + 2 * NT * C].rearrange(
                "p (i m) -> p i m", i=2)
            xpad8 = stg8[:, O8_XP:O8_XP + 2 * PH * PW].rearrange(
                "p (h i w) -> p i h w", i=2, w=PW)
            t_wsa = stg[:, O_WSA:O_WSA + MREP]
            t_wsb = stg[:, O_WSB:O_WSB + MREP]
            t_w1a = stg[:, O_W1A:O_W1A + 64]
            t_w1b = stg[:, O_W1B:O_W1B + 64]
            t_w2t = stg[0:64, O_W2T:O_W2T + 64]
            t_w3t = stg[0:64, O_W3T:O_W3T + NT * C]
            t_mc = stg[:, O_MC:O_MC + 18].bitcast(F32)  # [C, 9] f32
            xpad = stg[:, O_XP:O_XP + PH * PW].rearrange(
                "p (h w) -> p h w", w=PW)
            xpflat = stg[:, O_XP:O_XP + PH * PW]
            y2 = stg[:, O_Y2:O_Y2 + HW]
            t_wfa = stg[:, O_WFA:O_WFA + C]
            t_wfb = stg[:, O_WFB:O_WFB + C]
            t_bct = stg[:, O_BCT:O_BCT + NT * C]

            xs = S.tile([C, HW], BF16)
            sfs = S.tile([MREP, HW], BF16)

            xpart = S.tile([C, 4], F32)
            y2part = S.tile([C, 2], F32)
            sx = S.tile([C, 1], F32)
            y2sum = S.tile([C, 1], F32)
            edge = S.tile([C, 4], F32)   # R0, R63, C0, C63
            accs = S.tile([C, 8], F32)
            mxs = S.tile([C, 1], BF16)
            my2 = S.tile([C, 1], BF16)
            ctx1 = S.tile([64, 1], BF16)
            ctx2 = S.tile([64, 1], BF16)
            cfsb = S.tile([C, NT], F32)

            # ---------- input DMAs (few, large; need-ordered) ------------
            m8 = O8_XP + 18 * 2 * PW         # dsw8 + xp8 rows 0..18
            nc.sync.dma_start(out=stg8[:, 0:m8], in_=pk8[:, 0:m8])
            nc.sync.dma_start(out=stg8[:, m8:], in_=pk8[:, m8:])
            cut1 = O_Y2 + 2048               # wsab + first half of y2
            nc.sync.dma_start(out=stg[:, 0:cut1], in_=pk[:, 0:cut1])
            nc.sync.dma_start(out=stg[:, cut1:O_XP], in_=pk[:, cut1:O_XP])
            xh = O_XP + 33 * PW
            nc.sync.dma_start(out=stg[:, O_XP:xh], in_=pk[:, O_XP:xh])
            nc.sync.dma_start(out=stg[:, xh:O_W1A], in_=pk[:, xh:O_W1A])
            nc.sync.dma_start(out=stg[:, O_W1A:O_WFA], in_=pk[:, O_W1A:O_WFA])
            nc.sync.dma_start(out=stg[:, O_WFA:], in_=pk[:, O_WFA:])

            # ---------- means from X2/Y2 directly (DVE, overlaps A) ------
            HXP = 33 * PW
            for i in range(2):
                nc.vector.tensor_reduce(
                    out=xpart[:, i:i + 1],
                    in_=xpflat[:, HXP * i:HXP * (i + 1)], axis=AX.X, op=ADD)
            nc.vector.tensor_reduce(
                out=sx, in_=xpart[:, 0:2], axis=AX.X, op=ADD)
            for i in range(2):
                nc.vector.tensor_reduce(
                    out=y2part[:, i:i + 1],
                    in_=y2[:, 2048 * i:2048 * (i + 1)], axis=AX.X, op=ADD)
            nc.vector.tensor_reduce(out=y2sum, in_=y2part, axis=AX.X, op=ADD)
            nc.vector.tensor_scalar(
                out=my2, in0=y2sum, scalar1=1.0 / HW, scalar2=None, op0=MULT)
            # edge strips (row 0/63, col 0/63 of the unpadded image)
            nc.vector.tensor_reduce(
                out=edge[:, 0:1], in_=xpad[:, 1, 1:65], axis=AX.X, op=ADD)
            nc.vector.tensor_reduce(
                out=edge[:, 1:2], in_=xpad[:, 64, 1:65], axis=AX.X, op=ADD)
            nc.vector.tensor_reduce(
                out=edge[:, 2:3], in_=xpad[:, 1:65, 1], axis=AX.X, op=ADD)
            nc.vector.tensor_reduce(
                out=edge[:, 3:4], in_=xpad[:, 1:65, 64], axis=AX.X, op=ADD)
            # stt chain: mxs = sum_i coef_i * term_i  (coefs carry sign+1/HW)
            terms = [edge[:, 0:1], edge[:, 1:2], edge[:, 2:3], edge[:, 3:4],
                     xpad[:, 1, 1:2], xpad[:, 1, 64:65],
                     xpad[:, 64, 1:2], xpad[:, 64, 64:65]]
            nc.vector.tensor_scalar(
                out=accs[:, 0:1], in0=sx, scalar1=t_mc[:, 0:1], scalar2=None,
                op0=MULT)
            for i, term in enumerate(terms):
                dst = accs[:, i + 1:i + 2] if i < 7 else mxs
                nc.vector.scalar_tensor_tensor(
                    out=dst, in0=term, scalar=t_mc[:, i + 1:i + 2],
                    in1=accs[:, i:i + 1], op0=MULT, op1=ADD)

            # ---------- phase A: Xs (static depthwise) + sf --------------
            # PE program order is the execution order (in-order SEQ), so:
            # Xs0, Xs1, sf0, Xs2, sf1, [ctx], Xs3, sf2, sf3 - each sf(t)
            # waits on the ACT copy of xs(t), which runs under Xs(t+1).
            # The ctx branch slots into the tail where PE has slack; its
            # means inputs are DVE-computed straight from X2/Y2 above.
            def emit_xs(t, psA):
                xs_ps = psA.tile([C, 2, 512], F32, tag="xs_ps")
                for h in range(2):
                    for k in range(NT):
                        dh, dw = divmod(k, 3)
                        r0 = 16 * t + 8 * h + dh
                        rhs = xpad8[:, :, r0:r0 + 8, dw:dw + W]
                        nc.tensor.matmul(
                            xs_ps[:, h, :],
                            t_dsw8[:, :, k * C:(k + 1) * C],
                            rhs,
                            start=(k == 0),
                            stop=(k == NT - 1),
                            perf_mode=DROW,
                        )
                nc.scalar.copy(out=xs[:, t * STN:(t + 1) * STN], in_=xs_ps)

            def emit_sf(t, psSF):
                sf_ps = psSF.tile([MREP, 2, 512], F32, tag="sf_ps")
                _absorb(nc, xs[0:1, t * STN:t * STN + 1],
                        sf_ps[0:1, 0, 0:1])
                for h in range(2):
                    c0 = t * STN + h * 512
                    nc.tensor.matmul(
                        sf_ps[:, h, :], t_wsa, xs[:, c0:c0 + 512],
                        start=True, stop=False)
                    nc.tensor.matmul(
                        sf_ps[:, h, :], t_wsb, y2[:, c0:c0 + 512],
                        start=False, stop=True)
                nc.scalar.copy(
                    out=sfs[:, t * STN:(t + 1) * STN], in_=sf_ps)

            def emit_ctx(psX):
                ctx_ps = psX.tile([C, 11], F32, tag="ctx")
                ctx1_ps = ctx_ps[0:64, 0:1]
                ctx2_ps = ctx_ps[0:64, 1:2]
                cf_ps = ctx_ps[:, 2:11]
                _absorb(nc, mxs, ctx1_ps[0:1, 0:1])
                nc.tensor.matmul(ctx1_ps, t_w1a, mxs, start=True, stop=False)
                nc.tensor.matmul(ctx1_ps, t_w1b, my2, start=False, stop=True)
                nc.scalar.copy(out=ctx1, in_=ctx1_ps)

                nc.tensor.matmul(ctx2_ps, t_w2t, ctx1, start=True, stop=True)
                nc.scalar.activation(out=ctx2, in_=ctx2_ps, func=ACT_RELU)

                for k in range(NT):
                    nc.tensor.matmul(
                        cf_ps[:, k:k + 1], t_w3t[:, k * C:(k + 1) * C],
                        ctx2, start=True, stop=True)
                nc.scalar.copy(out=cfsb, in_=cf_ps)

            with tc.tile_pool(name="psA", bufs=2, space="PSUM") as psA, \
                 tc.tile_pool(name="psSF", bufs=1, space="PSUM") as psSF, \
                 tc.tile_pool(name="psCtx", bufs=1, space="PSUM") as psX:
                emit_xs(0, psA)
                emit_xs(1, psA)
                emit_sf(0, psSF)
                emit_xs(2, psA)
                emit_sf(1, psSF)
                emit_ctx(psX)
                emit_xs(3, psA)
                emit_sf(2, psSF)
                emit_sf(3, psSF)

            # ---------- phase C: dynamic filter + fusion conv ------------
            # Per tile, the PE stream is pipelined for lookahead: the three
            # Pool taps' broadcasts + ACT bias-copies are issued FIRST (their
            # producer chain is ~3us deep), their fusion matmuls run LAST;
            # the six DVE taps run in the middle with bc one tap ahead of
            # the fusion matmul. PSUM: psBC 2x2 + psO 2x2 = 8 banks.
            # 8 half-tiles of 512 px: 1 PSUM bank per bc / out tile gives a
            # deep psBC lookahead (bufs=5) so DVE/Pool producers never wait
            # on PE emission. Pool taps interleave with DVE taps at the tile
            # head; their fusion matmuls rejoin mid-accumulation-chain.
            obf = ob.rearrange("c h w -> c (h w)")
            with tc.tile_pool(name="psBC", bufs=5, space="PSUM") as psBC, \
                 tc.tile_pool(name="psOut", bufs=2, space="PSUM") as psO, \
                 tc.tile_pool(name="pP", bufs=8) as pP, \
                 tc.tile_pool(name="pDF", bufs=4) as pDF, \
                 tc.tile_pool(name="pOsb", bufs=2) as pOsb:
                DVE_TAPS = tuple(k for k in range(NT) if k not in ACT_TAPS)
                for u in range(2 * NST):
                    c0 = u * 512

                    def emit_bc(k, first=False):
                        g = k % 2
                        bc_ps = psBC.tile([C, 8, W], F32, tag="bc")
                        if first:
                            _absorb(nc, sfs[0:1, c0:c0 + 1],
                                    bc_ps[0:1, 0, 0:1])
                        nc.tensor.matmul(
                            bc_ps[:, :, :],
                            t_bct[32 * g:32 * g + NT, k * C:(k + 1) * C],
                            sfs[32 * g:32 * g + NT, c0:c0 + 512],
                            start=True, stop=True,
                            tile_position=(32 * g, 0),
                        )
                        return bc_ps

                    def xslice(k):
                        dh, dw = divmod(k, 3)
                        return xpad[:, 8 * u + dh:8 * u + dh + 8, dw:dw + W]

                    def emit_pool_tap(k, first=False):
                        bc_ps = emit_bc(k, first)
                        df = pDF.tile([C, 8, W], BF16, tag="df")
                        nc.scalar.activation(
                            out=df, in_=bc_ps, func=ACT_IDENT,
                            bias=cfsb[:, k:k + 1])
                        p_sb = pP.tile([C, 8, W], BF16, tag="p")
                        nc.gpsimd.tensor_tensor(
                            out=p_sb, in0=df, in1=xslice(k), op=MULT)
                        return p_sb

                    def emit_dve_tap(k):
                        bc_ps = emit_bc(k)
                        p_sb = pP.tile([C, 8, W], BF16, tag="p")
                        nc.vector.scalar_tensor_tensor(
                            out=p_sb, in0=bc_ps,
                            scalar=cfsb[:, k:k + 1], in1=xslice(k),
                            op0=ADD, op1=MULT)
                        return p_sb

                    def emit_wfb(p_sb, stop=False):
                        nc.tensor.matmul(
                            out_ps, t_wfb, p_sb,
                            start=False, stop=stop)

                    out_ps = psO.tile([C, 8, W], F32, tag="out_ps")
                    _absorb(nc, xs[0:1, c0:c0 + 1], out_ps[0:1, 0, 0:1])
                    nc.tensor.matmul(
                        out_ps, t_wfa, xs[:, c0:c0 + 512],
                        start=True, stop=False)
                    P = {}
                    P[ACT_TAPS[0]] = emit_pool_tap(ACT_TAPS[0], first=True)
                    P[DVE_TAPS[0]] = emit_dve_tap(DVE_TAPS[0])
                    P[ACT_TAPS[1]] = emit_pool_tap(ACT_TAPS[1])
                    P[DVE_TAPS[1]] = emit_dve_tap(DVE_TAPS[1])
                    emit_wfb(P[DVE_TAPS[0]])
                    P[ACT_TAPS[2]] = emit_pool_tap(ACT_TAPS[2])
                    P[DVE_TAPS[2]] = emit_dve_tap(DVE_TAPS[2])
                    emit_wfb(P[DVE_TAPS[1]])
                    P[DVE_TAPS[3]] = emit_dve_tap(DVE_TAPS[3])
                    emit_wfb(P[DVE_TAPS[2]])
                    emit_wfb(P[ACT_TAPS[0]])
                    P[DVE_TAPS[4]] = emit_dve_tap(DVE_TAPS[4])
                    emit_wfb(P[DVE_TAPS[3]])
                    emit_wfb(P[ACT_TAPS[1]])
                    P[DVE_TAPS[5]] = emit_dve_tap(DVE_TAPS[5])
                    emit_wfb(P[DVE_TAPS[4]])
                    emit_wfb(P[ACT_TAPS[2]])
                    emit_wfb(P[DVE_TAPS[5]], stop=True)
                    o_sb = pOsb.tile([C, 8, W], F32, tag="osb")
                    nc.scalar.copy(out=o_sb, in_=out_ps)
                    nc.sync.dma_start(
                        out=obf[:, c0:c0 + 512],
                        in_=o_sb.rearrange("c r w -> c (r w)"),
                    )
    _split_multiwaits(nc)
    return nc


def _prep_weights(static_w, w1, w2, w3, ws, wf):
    """Repack the tiny weights into the bf16 SBUF layouts."""
    f = np.float32
    sw = np.ascontiguousarray(static_w.reshape(C, NT), dtype=f)

    # fp8 DoubleRow depthwise weights: [64, 2, NT*C]
    dsw8 = np.zeros((64, 2, NT * C), dtype=f)
    for k in range(NT):
        for m in range(C):
            dsw8[m % 64, m // 64, k * C + m] = sw[m, k]
    dsw8 = dsw8.reshape(64, 2 * NT * C).astype(F8DT)

    wsa = np.zeros((C, MREP), dtype=f)
    wsb = np.zeros((C, MREP), dtype=f)
    for g in range(4):
        for k in range(NT):
            wsa[:, 32 * g + k] = ws[k, :C]
            wsb[:, 32 * g + k] = ws[k, C:]

    w1a = np.ascontiguousarray(w1[:, :C].T, dtype=f)
    w1b = np.ascontiguousarray(w1[:, C:].T, dtype=f)
    w2t64 = np.ascontiguousarray(w2.T, dtype=f)
    w3t64 = np.ascontiguousarray(
        w3.reshape(C, NT, 64).transpose(2, 1, 0), dtype=f).reshape(64, NT * C)
    w2t = np.zeros((C, 64), dtype=f)
    w2t[:64] = w2t64
    w3t = np.zeros((C, NT * C), dtype=f)
    w3t[:64] = w3t64

    # mean-correction coefficients (sign and 1/HW folded in):
    # terms: [S, R0, R63, C0, C63, X00, X0_63, X63_0, X63_63]
    mc = np.zeros((C, 9), dtype=f)
    mc[:, 0] = sw.sum(axis=1)
    mc[:, 1] = -(sw[:, 6] + sw[:, 7] + sw[:, 8])   # dh=+1 excludes row0
    mc[:, 2] = -(sw[:, 0] + sw[:, 1] + sw[:, 2])   # dh=-1 excludes row63
    mc[:, 3] = -(sw[:, 2] + sw[:, 5] + sw[:, 8])   # dw=+1 excludes col0
    mc[:, 4] = -(sw[:, 0] + sw[:, 3] + sw[:, 6])   # dw=-1 excludes col63
    mc[:, 5] = sw[:, 8]
    mc[:, 6] = sw[:, 6]
    mc[:, 7] = sw[:, 2]
    mc[:, 8] = sw[:, 0]
    mc *= 1.0 / HW
    mc_bf = np.ascontiguousarray(mc).view(np.uint16).view(BFDT)  # [C, 18]

    wfa = np.ascontiguousarray(wf[:, :C].T, dtype=f)
    wfb = np.ascontiguousarray(wf[:, C:].T, dtype=f)

    bct = np.zeros((C, NT * C), dtype=f)
    for g in range(4):
        for k in range(NT):
            bct[32 * g + k, k * C:(k + 1) * C] = 1.0

    wts_pre = np.concatenate([wsa, wsb], axis=1).astype(BFDT)
    wts_mid = np.concatenate(
        [w1a, w1b, w2t, w3t], axis=1).astype(BFDT)
    wts_mid = np.concatenate([wts_mid, mc_bf], axis=1)
    wts_post = np.concatenate([wfa, wfb, bct], axis=1).astype(BFDT)
    return wts_pre, wts_mid, wts_post, dsw8


def make_in_maps(X2, Y2, static_w, w1, w2, w3, ws, wf):
    wts_pre, wts_mid, wts_post, dsw8 = _prep_weights(
        np.asarray(static_w), np.asarray(w1), np.asarray(w2),
        np.asarray(w3), np.asarray(ws), np.asarray(wf),
    )
    X2 = np.asarray(X2)
    Y2 = np.asarray(Y2)
    xpad_f = np.zeros((B, C, PH, PW), dtype=np.float32)
    xpad_f[:, :, 1:H + 1, 1:W + 1] = X2
    xpad_all = xpad_f.reshape(B, C, PH * PW).astype(BFDT)
    # [B, 64, PH, 2, PW]: row-major, channel-half pairs interleaved per row
    xp8_all = xpad_f.reshape(B, 2, 64, PH, PW).transpose(0, 2, 3, 1, 4).reshape(
        B, 64, 2 * PH * PW).astype(F8DT)
    y2_all = Y2.reshape(B, C, HW).astype(BFDT)
    in_maps = []
    for b in range(B):
        m = {"pk": np.ascontiguousarray(np.concatenate(
                [wts_pre, y2_all[b], xpad_all[b], wts_mid, wts_post],
                axis=1)),
             "pk8": np.ascontiguousarray(np.concatenate(
                [dsw8, xp8_all[b]], axis=1))}
        in_maps.append(m)
    return in_maps


def get_nc():
    if "nc" not in _CACHE:
        _CACHE["nc"] = _build_bass()
    return _CACHE["nc"]


def kernel(X2, Y2, static_w, w1, w2, w3, ws, wf):
    nc = get_nc()
    in_maps = make_in_maps(
        np.asarray(X2), np.asarray(Y2), static_w, w1, w2, w3, ws, wf
    )
    res = run_bass_kernel_spmd(nc, in_maps, core_ids=list(range(B)))
    out = np.stack([np.asarray(r["ob"], dtype=np.float32)
                    for r in res.results])
    return out


# revision 28
# speedup vs baseline: 1.9184x; 1.1090x over previous
"""Trainium2 Bass kernel for the CMDF block (dense_cnn), V3.

Contract: kernel(**inputs) takes the FULL unsharded inputs (B=8, C=128,
H=W=64) and returns the FULL (8, 128, 64, 64) float32 output.

Sharding: data-parallel over batch - core b computes batch element b.
All weights are replicated (host-side prepacked into matmul layouts).

Math per batch element (see reference):
  Xs   = depthwise3x3(X2, static_w)
  ctx  = relu(w2 @ (w1 @ mean_hw([Xs; Y2])))
  cf   = (w3 @ ctx).reshape(C, 9)          # per-channel dynamic filter
  sf   = ws @ [Xs; Y2]                     # (9, H, W) spatial filter
  dyn  = sum_k shift_k(X2) * (cf[:, k] + sf[k])
  out  = wf[:, :C] @ Xs + wf[:, C:] @ dyn

V3 design notes (vs the f32r V1):
  - Everything streams in bf16 (3.2MB input vs 6.4MB): DMA lead-in halves.
    Weights land first (chunked DMA) so PE starts at ~3us, not ~21us.
  - mean_hw(Xs) is computed WITHOUT Xs: mean of a zero-padded shifted
    image = S - edge strips + corners, so a per-tap host-precomputed
    coefficient vector turns 9 shifted means into one stt chain over
    {S, row0, row63, col0, col63, 4 corners}. The context branch then
    runs concurrently with phase A instead of serializing after it.
  - Phase C taps are split across engines: 6 taps as DVE stt reading the
    broadcast filter from PSUM; 3 taps via ACT Identity-with-bias copy
    (PSUM -> SBUF bf16, folds cf in) followed by a Pool tensor_tensor
    multiply. PE does the 9 broadcast + 9 fusion matmuls per tile and is
    the ~8.1us/tile bottleneck; DVE/ACT/Pool hide underneath.
"""

import numpy as np
import ml_dtypes

import concourse.bass as bass
import concourse.tile as tile
import concourse.mybir as mybir
from concourse.bass_utils import run_bass_kernel_spmd

B, C, H, W, K = 8, 128, 64, 64, 3
HW = H * W            # 4096
PH, PW = H + 2, W + 2  # 66, 66 padded
NST = 4               # super-tiles over rows
ROWS = H // NST       # 16 image rows per super-tile
STN = ROWS * W        # 1024 pixels per super-tile (2 PSUM banks)
NT = K * K            # 9 taps
MREP = 3 * 32 + NT    # 105: ws replicated at partition groups 0,32,64,96

F32 = mybir.dt.float32
BF16 = mybir.dt.bfloat16
FP8 = mybir.dt.float8e4
DROW = mybir.MatmulPerfMode.DoubleRow
ADD = mybir.AluOpType.add
MULT = mybir.AluOpType.mult
AX = mybir.AxisListType
ACT_COPY = mybir.ActivationFunctionType.Copy
ACT_IDENT = mybir.ActivationFunctionType.Identity
ACT_RELU = mybir.ActivationFunctionType.Relu

BFDT = ml_dtypes.bfloat16
F8DT = ml_dtypes.float8_e4m3fn

# ---- pk8 (fp8) column layout: DoubleRow channel-paired, 64 partitions ----
O8_DSW = 0                      # [64, 2, NT*C] depthwise diag
O8_XP = 2 * NT * C              # 2304; [64, PH, 2, PW] padded X2 (row-major)
PK8_COLS = O8_XP + 2 * PH * PW  # 11016

# taps handled by ACT(bias)+Pool(mult); the rest are DVE stt
ACT_TAPS = (0, 1, 2)

# ---- pk column layout (bf16 columns) ----
O_WSA = 0
O_WSB = O_WSA + MREP          # 105
O_Y2 = O_WSB + MREP           # 210
O_XP = O_Y2 + HW
O_W1A = O_XP + PH * PW
O_W1B = O_W1A + 64
O_W2T = O_W1B + 64
O_W3T = O_W2T + 64
O_MC = O_W3T + NT * C         # (9 f32 coeffs = 18 bf16 cols)
O_WFA = O_MC + 18
O_WFB = O_WFA + C
O_BCT = O_WFB + C
PK_COLS = O_BCT + NT * C

_CACHE = {}


def _absorb(nc, dep_elem, ps_elem):
    """Tiny bf16 matmul that reads one element of `dep_elem` and writes a
    junk element of `ps_elem` (later overwritten by a start=True group).
    Purpose: acquire the semaphore wait on dep_elem's producer on a plain
    (non-fused) matmul, so the following fused matmul - which can embed
    only ONE sem wait - doesn't need two."""
    lh = dep_elem
    nc.tensor.matmul(ps_elem, lh[:, 0:1], lh[:, 0:1], start=True, stop=True)


def _split_multiwaits(nc):
    """walrus codegen in this toolchain accepts only ONE embedded sem wait
    per instruction. Hoist excess waits onto same-engine NoOps placed
    immediately before the instruction (engines execute in order, so the
    blocking behavior is identical)."""
    ctr = 0
    for fn in nc.m.functions:
        for blk in fn.blocks:
            insts = blk.instructions
            out = []
            for inst in insts:
                si = inst.sync_info
                waits = list(si.on_wait) if si is not None and si.on_wait else []
                if len(waits) > 1:
                    for w in waits[:-1]:
                        ctr += 1
                        out.append(mybir.InstNoOp(
                            name=f"I-wsplit-{ctr}",
                            engine=inst.engine,
                            ins=[], outs=[],
                            sync_info=mybir.SyncInfo(
                                on_wait=[w], on_update=[]),
                        ))
                    inst.sync_info = mybir.SyncInfo(
                        on_wait=[waits[-1]],
                        on_update=list(si.on_update) if si.on_update else [],
                    )
                out.append(inst)
            blk.instructions = out


def _build_bass():
    nc = bass.Bass("TRN2", target_bir_lowering=False, debug=False)

    pk = nc.dram_tensor("pk", [C, PK_COLS], BF16, kind="ExternalInput").ap()
    pk8 = nc.dram_tensor("pk8", [64, PK8_COLS], FP8, kind="ExternalInput").ap()
    ob = nc.dram_tensor("ob", [C, H, W], F32, kind="ExternalOutput").ap()

    with tile.TileContext(nc) as tc:
        with tc.tile_pool(name="singles", bufs=1) as S:
            stg = S.tile([C, PK_COLS], BF16)
            stg8 = S.tile([64, PK8_COLS], FP8)
            t_dsw8 = stg8[:, O8_DSW:O8_DSW + 2 * NT * C].rearrange(
                "p (i m) -> p i m", i=2)
            xpad8 = stg8[:, O8_XP:O8_XP + 2 * PH * PW].rearrange(
                "p (h i w) -> p i h w", i=2, w=PW)
            t_wsa = stg[:, O_WSA:O_WSA + MREP]
            t_wsb = stg[:, O_WSB:O_WSB + MREP]
            t_w1a = stg[:, O_W1A:O_W1A + 64]
            t_w1b = stg[:, O_W1B:O_W1B + 64]
            t_w2t = stg[0:64, O_W2T:O_W2T + 64]
            t_w3t = stg[0:64, O_W3T:O_W3T + NT * C]
            t_mc = stg[:, O_MC:O_MC + 18].bitcast(F32)  # [C, 9] f32
            xpad = stg[:, O_XP:O_XP + PH * PW].rearrange(
                "p (h w) -> p h w", w=PW)
            xpflat = stg[:, O_XP:O_XP + PH * PW]
            y2 = stg[:, O_Y2:O_Y2 + HW]
            t_wfa = stg[:, O_WFA:O_WFA + C]
            t_wfb = stg[:, O_WFB:O_WFB + C]
            t_bct = stg[:, O_BCT:O_BCT + NT * C]

            xs = S.tile([C, HW], BF16)
            sfs = S.tile([MREP, HW], BF16)

            xs_parts = S.tile([C, 8], F32)
            y2part = S.tile([C, 2], F32)
            sx = S.tile([C, 1], F32)
            y2sum = S.tile([C, 1], F32)
            mxs = S.tile([C, 1], BF16)
            my2 = S.tile([C, 1], BF16)
            ctx1 = S.tile([64, 1], BF16)
            ctx2 = S.tile([64, 1], BF16)
            cfsb = S.tile([C, NT], F32)

            # ---------- PE warmup: ramp the p-state before data lands ----
            # The cost model's tensor-engine clock ramps with sustained
            # execution (0.65 -> 1.2 -> 2.4 GHz after ~3us). Spin dummy
            # matmuls on a memset tile so the real phase-A matmuls start
            # at full clock instead of paying the ramp.
            wtile = S.tile([C, 512], BF16)
            nc.gpsimd.memset(wtile, 0.0)

            # ---------- input DMAs (few, large; need-ordered) ------------
            m8 = O8_XP + 18 * 2 * PW         # dsw8 + xp8 rows 0..18
            nc.sync.dma_start(out=stg8[:, 0:m8], in_=pk8[:, 0:m8])
            nc.sync.dma_start(out=stg8[:, m8:], in_=pk8[:, m8:])
            cut1 = O_Y2 + 2048               # wsab + first half of y2
            nc.sync.dma_start(out=stg[:, 0:cut1], in_=pk[:, 0:cut1])
            nc.sync.dma_start(out=stg[:, cut1:O_XP], in_=pk[:, cut1:O_XP])
            xh = O_XP + 33 * PW
            nc.sync.dma_start(out=stg[:, O_XP:xh], in_=pk[:, O_XP:xh])
            nc.sync.dma_start(out=stg[:, xh:O_W1A], in_=pk[:, xh:O_W1A])
            nc.sync.dma_start(out=stg[:, O_W1A:O_WFA], in_=pk[:, O_W1A:O_WFA])
            nc.sync.dma_start(out=stg[:, O_WFA:], in_=pk[:, O_WFA:])

            # ---------- means (DVE y2 reduces; mean(Xs) via ACT accum) ---
            nc.vector.tensor_reduce(
                out=y2part[:, 0:1], in_=y2[:, 0:2048], axis=AX.X, op=ADD)
            nc.vector.tensor_reduce(
                out=y2part[:, 1:2], in_=y2[:, 2048:4096], axis=AX.X, op=ADD)
            nc.vector.tensor_reduce(out=y2sum, in_=y2part, axis=AX.X, op=ADD)
            nc.vector.tensor_scalar(
                out=my2, in0=y2sum, scalar1=1.0 / HW, scalar2=None, op0=MULT)

            # ---------- phase A: Xs (static depthwise) + sf --------------
            # PE program order is the execution order (in-order SEQ), so:
            # Xs0, Xs1, sf0, Xs2, sf1, [ctx], Xs3, sf2, sf3 - each sf(t)
            # waits on the ACT copy of xs(t), which runs under Xs(t+1).
            # The ctx branch slots into the tail where PE has slack; its
            # means inputs are DVE-computed straight from X2/Y2 above.
            def emit_xs(u, psA):
                xs_ps = psA.tile([C, 512], F32, tag="xs_ps")
                for k in range(NT):
                    dh, dw = divmod(k, 3)
                    r0 = 8 * u + dh
                    rhs = xpad8[:, :, r0:r0 + 8, dw:dw + W]
                    nc.tensor.matmul(
                        xs_ps,
                        t_dsw8[:, :, k * C:(k + 1) * C],
                        rhs,
                        start=(k == 0),
                        stop=(k == NT - 1),
                        perf_mode=DROW,
                    )
                nc.scalar.activation(
                    out=xs[:, u * 512:(u + 1) * 512], in_=xs_ps,
                    func=ACT_COPY, accum_out=xs_parts[:, u:u + 1])

            def emit_sf(u, psSF):
                sf_ps = psSF.tile([MREP, 512], F32, tag="sf_ps")
                _absorb(nc, xs[0:1, u * 512:u * 512 + 1],
                        sf_ps[0:1, 0:1])
                c0 = u * 512
                nc.tensor.matmul(
                    sf_ps, t_wsa, xs[:, c0:c0 + 512],
                    start=True, stop=False)
                nc.tensor.matmul(
                    sf_ps, t_wsb, y2[:, c0:c0 + 512],
                    start=False, stop=True)
                if u < 4:
                    nc.scalar.copy(out=sfs[:, c0:c0 + 512], in_=sf_ps)
                else:
                    # late blocks: DVE does the PSUM->SBUF copy so the ACT
                    # chain (which gates mean(Xs) -> ctx -> cfsb) ends early
                    nc.vector.tensor_copy(out=sfs[:, c0:c0 + 512], in_=sf_ps)

            def emit_mxs():
                nc.vector.tensor_reduce(
                    out=sx, in_=xs_parts, axis=AX.X, op=ADD)
                nc.vector.tensor_scalar(
                    out=mxs, in0=sx, scalar1=1.0 / HW, scalar2=None, op0=MULT)

            def emit_ctx(psX):
                ctx_ps = psX.tile([C, 11], F32, tag="ctx")
                ctx1_ps = ctx_ps[0:64, 0:1]
                ctx2_ps = ctx_ps[0:64, 1:2]
                cf_ps = ctx_ps[:, 2:11]
                _absorb(nc, mxs, ctx1_ps[0:1, 0:1])
                nc.tensor.matmul(ctx1_ps, t_w1a, mxs, start=True, stop=False)
                nc.tensor.matmul(ctx1_ps, t_w1b, my2, start=False, stop=True)
                nc.scalar.copy(out=ctx1, in_=ctx1_ps)

                nc.tensor.matmul(ctx2_ps, t_w2t, ctx1, start=True, stop=True)
                nc.scalar.activation(out=ctx2, in_=ctx2_ps, func=ACT_RELU)

                for k in range(NT):
                    nc.tensor.matmul(
                        cf_ps[:, k:k + 1], t_w3t[:, k * C:(k + 1) * C],
                        ctx2, start=True, stop=True)
                nc.scalar.copy(out=cfsb, in_=cf_ps)

            with tc.tile_pool(name="psA", bufs=2, space="PSUM") as psA, \
                 tc.tile_pool(name="psSF", bufs=2, space="PSUM") as psSF, \
                 tc.tile_pool(name="psCtx", bufs=1, space="PSUM") as psX:
                warm_ps = psX.tile([C, 512], F32, tag="warm")
                for _ in range(8):
                    nc.tensor.matmul(
                        warm_ps, wtile[:, 0:C], wtile,
                        start=True, stop=True)
                emit_xs(0, psA)
                emit_xs(1, psA)
                emit_sf(0, psSF)
                emit_xs(2, psA)
                emit_sf(1, psSF)
                emit_xs(3, psA)
                emit_sf(2, psSF)
                emit_xs(4, psA)
                emit_sf(3, psSF)
                emit_xs(5, psA)
                emit_sf(4, psSF)
                emit_xs(6, psA)
                emit_sf(5, psSF)
                emit_xs(7, psA)
                emit_sf(6, psSF)
                emit_mxs()
                emit_sf(7, psSF)
                emit_ctx(psX)

            # ---------- phase C: dynamic filter + fusion conv ------------
            # Per tile, the PE stream is pipelined for lookahead: the three
            # Pool taps' broadcasts + ACT bias-copies are issued FIRST (their
            # producer chain is ~3us deep), their fusion matmuls run LAST;
            # the six DVE taps run in the middle with bc one tap ahead of
            # the fusion matmul. PSUM: psBC 2x2 + psO 2x2 = 8 banks.
            # 8 half-tiles of 512 px: 1 PSUM bank per bc / out tile gives a
            # deep psBC lookahead (bufs=5) so DVE/Pool producers never wait
            # on PE emission. Pool taps interleave with DVE taps at the tile
            # head; their fusion matmuls rejoin mid-accumulation-chain.
            obf = ob.rearrange("c h w -> c (h w)")
            with tc.tile_pool(name="psBC", bufs=5, space="PSUM") as psBC, \
                 tc.tile_pool(name="psOut", bufs=2, space="PSUM") as psO, \
                 tc.tile_pool(name="pP", bufs=8) as pP, \
                 tc.tile_pool(name="pDF", bufs=4) as pDF, \
                 tc.tile_pool(name="pOsb", bufs=2) as pOsb:
                DVE_TAPS = tuple(k for k in range(NT) if k not in ACT_TAPS)
                for u in range(2 * NST):
                    c0 = u * 512

                    def emit_bc(k, first=False):
                        g = k % 2
                        bc_ps = psBC.tile([C, 8, W], F32, tag="bc")
                        if first:
                            _absorb(nc, sfs[0:1, c0:c0 + 1],
                                    bc_ps[0:1, 0, 0:1])
                        nc.tensor.matmul(
                            bc_ps[:, :, :],
                            t_bct[32 * g:32 * g + NT, k * C:(k + 1) * C],
                            sfs[32 * g:32 * g + NT, c0:c0 + 512],
                            start=True, stop=True,
                            tile_position=(32 * g, 0),
                        )
                        return bc_ps

                    def xslice(k):
                        dh, dw = divmod(k, 3)
                        return xpad[:, 8 * u + dh:8 * u + dh + 8, dw:dw + W]

                    def emit_pool_tap(k, first=False):
                        bc_ps = emit_bc(k, first)
                        df = pDF.tile([C, 8, W], BF16, tag="df")
                        nc.scalar.activation(
                            out=df, in_=bc_ps, func=ACT_IDENT,
                            bias=cfsb[:, k:k + 1])
                        p_sb = pP.tile([C, 8, W], BF16, tag="p")
                        nc.gpsimd.tensor_tensor(
                            out=p_sb, in0=df, in1=xslice(k), op=MULT)
                        return p_sb

                    def emit_dve_tap(k):
                        bc_ps = emit_bc(k)
                        p_sb = pP.tile([C, 8, W], BF16, tag="p")
                        nc.vector.scalar_tensor_tensor(
                            out=p_sb, in0=bc_ps,
                            scalar=cfsb[:, k:k + 1], in1=xslice(k),
                            op0=ADD, op1=MULT)
                        return p_sb

                    def emit_wfb(p_sb, stop=False):
                        nc.tensor.matmul(
                            out_ps, t_wfb, p_sb,
                            start=False, stop=stop)

                    out_ps = psO.tile([C, 8, W], F32, tag="out_ps")
                    _absorb(nc, xs[0:1, c0:c0 + 1], out_ps[0:1, 0, 0:1])
                    nc.tensor.matmul(
                        out_ps, t_wfa, xs[:, c0:c0 + 512],
                        start=True, stop=False)
                    P = {}
                    P[ACT_TAPS[0]] = emit_pool_tap(ACT_TAPS[0], first=True)
                    P[DVE_TAPS[0]] = emit_dve_tap(DVE_TAPS[0])
                    P[ACT_TAPS[1]] = emit_pool_tap(ACT_TAPS[1])
                    P[DVE_TAPS[1]] = emit_dve_tap(DVE_TAPS[1])
                    emit_wfb(P[DVE_TAPS[0]])
                    P[ACT_TAPS[2]] = emit_pool_tap(ACT_TAPS[2])
                    P[DVE_TAPS[2]] = emit_dve_tap(DVE_TAPS[2])
                    emit_wfb(P[DVE_TAPS[1]])
                    P[DVE_TAPS[3]] = emit_dve_tap(DVE_TAPS[3])
                    emit_wfb(P[DVE_TAPS[2]])
                    emit_wfb(P[ACT_TAPS[0]])
                    P[DVE_TAPS[4]] = emit_dve_tap(DVE_TAPS[4])
                    emit_wfb(P[DVE_TAPS[3]])
                    emit_wfb(P[ACT_TAPS[1]])
                    P[DVE_TAPS[5]] = emit_dve_tap(DVE_TAPS[5])
                    emit_wfb(P[DVE_TAPS[4]])
                    emit_wfb(P[ACT_TAPS[2]])
                    emit_wfb(P[DVE_TAPS[5]], stop=True)
                    o_sb = pOsb.tile([C, 8, W], F32, tag="osb")
                    nc.scalar.copy(out=o_sb, in_=out_ps)
                    nc.sync.dma_start(
                        out=obf[:, c0:c0 + 512],
                        in_=o_sb.rearrange("c r w -> c (r w)"),
                    )
    _split_multiwaits(nc)
    return nc


def _prep_weights(static_w, w1, w2, w3, ws, wf):
    """Repack the tiny weights into the bf16 SBUF layouts."""
    f = np.float32
    sw = np.ascontiguousarray(static_w.reshape(C, NT), dtype=f)

    # fp8 DoubleRow depthwise weights: [64, 2, NT*C]
    dsw8 = np.zeros((64, 2, NT * C), dtype=f)
    for k in range(NT):
        for m in range(C):
            dsw8[m % 64, m // 64, k * C + m] = sw[m, k]
    dsw8 = dsw8.reshape(64, 2 * NT * C).astype(F8DT)

    wsa = np.zeros((C, MREP), dtype=f)
    wsb = np.zeros((C, MREP), dtype=f)
    for g in range(4):
        for k in range(NT):
            wsa[:, 32 * g + k] = ws[k, :C]
            wsb[:, 32 * g + k] = ws[k, C:]

    w1a = np.ascontiguousarray(w1[:, :C].T, dtype=f)
    w1b = np.ascontiguousarray(w1[:, C:].T, dtype=f)
    w2t64 = np.ascontiguousarray(w2.T, dtype=f)
    w3t64 = np.ascontiguousarray(
        w3.reshape(C, NT, 64).transpose(2, 1, 0), dtype=f).reshape(64, NT * C)
    w2t = np.zeros((C, 64), dtype=f)
    w2t[:64] = w2t64
    w3t = np.zeros((C, NT * C), dtype=f)
    w3t[:64] = w3t64

    # mean-correction coefficients (sign and 1/HW folded in):
    # terms: [S, R0, R63, C0, C63, X00, X0_63, X63_0, X63_63]
    mc = np.zeros((C, 9), dtype=f)
    mc[:, 0] = sw.sum(axis=1)
    mc[:, 1] = -(sw[:, 6] + sw[:, 7] + sw[:, 8])   # dh=+1 excludes row0
    mc[:, 2] = -(sw[:, 0] + sw[:, 1] + sw[:, 2])   # dh=-1 excludes row63
    mc[:, 3] = -(sw[:, 2] + sw[:, 5] + sw[:, 8])   # dw=+1 excludes col0
    mc[:, 4] = -(sw[:, 0] + sw[:, 3] + sw[:, 6])   # dw=-1 excludes col63
    mc[:, 5] = sw[:, 8]
    mc[:, 6] = sw[:, 6]
    mc[:, 7] = sw[:, 2]
    mc[:, 8] = sw[:, 0]
    mc *= 1.0 / HW
    mc_bf = np.ascontiguousarray(mc).view(np.uint16).view(BFDT)  # [C, 18]

    wfa = np.ascontiguousarray(wf[:, :C].T, dtype=f)
    wfb = np.ascontiguousarray(wf[:, C:].T, dtype=f)

    bct = np.zeros((C, NT * C), dtype=f)
    for g in range(4):
        for k in range(NT):
            bct[32 * g + k, k * C:(k + 1) * C] = 1.0

    wts_pre = np.concatenate([wsa, wsb], axis=1).astype(BFDT)
    wts_mid = np.concatenate(
        [w1a, w1b, w2t, w3t], axis=1).astype(BFDT)
    wts_mid = np.concatenate([wts_mid, mc_bf], axis=1)
    wts_post = np.concatenate([wfa, wfb, bct], axis=1).astype(BFDT)
    return wts_pre, wts_mid, wts_post, dsw8


def make_in_maps(X2, Y2, static_w, w1, w2, w3, ws, wf):
    wts_pre, wts_mid, wts_post, dsw8 = _prep_weights(
        np.asarray(static_w), np.asarray(w1), np.asarray(w2),
        np.asarray(w3), np.asarray(ws), np.asarray(wf),
    )
    X2 = np.asarray(X2)
    Y2 = np.asarray(Y2)
    xpad_f = np.zeros((B, C, PH, PW), dtype=np.float32)
    xpad_f[:, :, 1:H + 1, 1:W + 1] = X2
    xpad_all = xpad_f.reshape(B, C, PH * PW).astype(BFDT)
    # [B, 64, PH, 2, PW]: row-major, channel-half pairs interleaved per row
    xp8_all = xpad_f.reshape(B, 2, 64, PH, PW).transpose(0, 2, 3, 1, 4).reshape(
        B, 64, 2 * PH * PW).astype(F8DT)
    y2_all = Y2.reshape(B, C, HW).astype(BFDT)
    in_maps = []
    for b in range(B):
        m = {"pk": np.ascontiguousarray(np.concatenate(
                [wts_pre, y2_all[b], xpad_all[b], wts_mid, wts_post],
                axis=1)),
             "pk8": np.ascontiguousarray(np.concatenate(
                [dsw8, xp8_all[b]], axis=1))}
        in_maps.append(m)
    return in_maps


def get_nc():
    if "nc" not in _CACHE:
        _CACHE["nc"] = _build_bass()
    return _CACHE["nc"]


def kernel(X2, Y2, static_w, w1, w2, w3, ws, wf):
    nc = get_nc()
    in_maps = make_in_maps(
        np.asarray(X2), np.asarray(Y2), static_w, w1, w2, w3, ws, wf
    )
    res = run_bass_kernel_spmd(nc, in_maps, core_ids=list(range(B)))
    out = np.stack([np.asarray(r["ob"], dtype=np.float32)
                    for r in res.results])
    return out
